# revision 55
# baseline (speedup 1.0000x reference)
"""Trainium2 Bass kernel for nn_JointRelationModule (self-contained).

Math (per person p; softmax is segment-softmax over persons within an imgid
group, elementwise over the (K,K) score entries):
    q = Wq x + bq ; k = Wk x + bk ; v = Wv x + bv      (1x1 conv over K=17)
    S_p = q_p k_p^T / 64
    attn = segment-softmax over persons
    out = relu(attn_p @ v_p + x_p)

Device formulation (heavy ops bf16 on the PE, block-column layouts):
  - Stack BD=7 persons as [119, hw]. Per stack: G = x x^T via PE transpose +
    accumulating matmuls (bf16, f32 PSUM).
  - scores^T in block-column layout [119, 17] via a masked-Gram matmul chain
    (block-diag mask kills cross-person terms), so no gather/scatter DMAs.
  - Segment softmax via per-stack selector matmuls into group-slot tiles,
    reciprocal, selector-transpose broadcast back; all partition-aligned.
  - Output: B = blockdiag((attn Wv)^T) + I with an av row appended; the
    residual and v-bias ride along x_aug (all-ones row), so each output chunk
    is one matmul + one relu. B is zero-padded to 128 weight columns so the
    PE fast-weight-load path kicks in. Stored bf16, host upcasts.

Data movement: x and y live in a partition-major layout [120, S*hw]. All bulk
x loads / y stores ride the gpsimd (SWDGE) ring: its descriptors spread
evenly over all 16 SDMA engines, unlike the HWDGE (sync/scalar) rings which
concentrate on engines 0-6. Small constants ride the otherwise-idle HWDGE
rings. Stack 0's load is column-chunked so the first transposes start early.

Sharding: data-parallel over persons at imgid group boundaries (8 cores),
weights replicated. Host casts x to bf16 (halves load bytes); output comes
back bf16 (halves store bytes). Tolerance 2e-2; measured error ~5e-3.
"""

import math
import sys

import numpy as np

K = 17
HW = 4096  # 64*64
P_TOTAL = 512
N_CORES = 8
NORM = 64.0
BD = 7          # persons per stack
BDK = BD * K    # 119
O_CH = 512      # output chunk cols (one PSUM bank of f32)
WPAD = 128      # output-matmul weight cols padded for fast weight load

_cache: dict = {}


def _ensure_path():
    try:
        import concourse.bass  # noqa: F401
    except ImportError:
        for p in ("/opt/trn_rl_repo", "/root/.axon_site/_ro/trn_rl_repo"):
            if p not in sys.path:
                sys.path.insert(0, p)
        import concourse.bass  # noqa: F401


def _build(S: int, T: int, have_bias: bool, used: tuple):
    """Builds + compiles the per-core SPMD Bass program.

    Sliding-window softmax: group-slot tiles hold only a few groups each, so
    tile t's reciprocal is ready as soon as its last contributing stack's exp
    is done; output stacks trail input stacks by the tile span (~3 stacks).
    Loads and stores interleave continuously on the SWDGE ring."""
    _ensure_path()
    import concourse.bacc as bacc
    import concourse.mybir as mybir
    import concourse.tile as tile

    f32 = mybir.dt.float32
    bf16 = mybir.dt.bfloat16
    Exp = mybir.ActivationFunctionType.Exp
    Relu = mybir.ActivationFunctionType.Relu

    U = len(used)
    tiles_of = {s: sorted({t for (ss, t) in used if ss == s})
                for s in range(S)}
    ready = {t: max(ss for (ss, tt) in used if tt == t)
             for t in range(T)}

    nc = bacc.Bacc(
        "TRN2",
        target_bir_lowering=False,
        debug=False,
        enable_asserts=False,
        num_devices=N_CORES,
    )

    x_d = nc.dram_tensor("x", [BDK + 1, S * HW], bf16, kind="ExternalInput")
    wq_d = nc.dram_tensor("wq_col", [BDK, K], f32, kind="ExternalInput")
    wk_d = nc.dram_tensor("wkt_bd", [BDK, BDK], f32, kind="ExternalInput")
    wv_d = nc.dram_tensor("wv_aug", [BDK, BDK + 1], bf16, kind="ExternalInput")
    id_d = nc.dram_tensor("id119", [BDK, BDK], bf16, kind="ExternalInput")
    ia_d = nc.dram_tensor("iaug", [BDK + 1, BDK], f32, kind="ExternalInput")
    mk_d = nc.dram_tensor("bdmask", [BDK, BDK], f32, kind="ExternalInput")
    mkb_d = nc.dram_tensor("bdmaskb", [BDK, BDK], bf16, kind="ExternalInput")
    sel_d = nc.dram_tensor("sel", [BDK, U * BDK], bf16, kind="ExternalInput")
    selt_d = nc.dram_tensor("selT", [BDK, U * BDK], bf16,
                            kind="ExternalInput")
    if have_bias:
        corr_d = nc.dram_tensor("corr_col", [BDK, K * S], f32,
                                kind="ExternalInput")
    y_d = nc.dram_tensor("y", [BDK, S * HW], bf16, kind="ExternalOutput")

    G_CH = 1024          # x cols per transpose group
    n_grp = HW // G_CH   # 4 groups per stack

    with tile.TileContext(nc) as tc:
        with (
            nc.allow_low_precision(reason="bf16 softmax ok at 2e-2 tol"),
            tc.tile_pool(name="xpool", bufs=1) as xpool,
            tc.tile_pool(name="cpool", bufs=1) as cpool,
            tc.tile_pool(name="wpool", bufs=2) as wpool,
            tc.tile_pool(name="opool", bufs=2) as opool,
            tc.tile_pool(name="pp", bufs=2, space="PSUM") as pp,
        ):
            # --- tiny phase-A constants first (id_t gates every transpose);
            # consts ride the HWDGE rings, bulk x/y rides the SWDGE ring ---
            id_t = cpool.tile([BDK, BDK], bf16, name="id_t", tag="id")
            mk_t = cpool.tile([BDK, BDK], f32, name="mk_t", tag="mk")
            wq_t = cpool.tile([BDK, K], f32, name="wq_t", tag="wq")
            wk_t = cpool.tile([BDK, BDK], f32, name="wk_t", tag="wk")
            nc.sync.dma_start(id_t[:], id_d.ap())

            x_tiles = []  # per stack

            def load_xtile(s):
                xt_ = xpool.tile([BDK + 1, HW], bf16, name=f"xp{s}",
                                 tag=f"xp{s}")
                base = s * HW
                # chunked so transposes can start on a partial tile: the
                # first chunk's arrival, not the whole tile's, gates compute
                nch = 4 if s == 0 else 2
                cw = HW // nch
                for ci_ in range(nch):
                    nc.gpsimd.dma_start(
                        xt_[:, cw * ci_:cw * (ci_ + 1)],
                        x_d.ap()[:, base + cw * ci_:base + cw * (ci_ + 1)])
                x_tiles.append(xt_)

            nc.scalar.dma_start(mk_t[:], mk_d.ap())
            nc.sync.dma_start(wq_t[:], wq_d.ap())
            nc.scalar.dma_start(wk_t[:], wk_d.ap())
            load_xtile(0)
            if have_bias:
                corr_t = cpool.tile([BDK, K * S], f32, name="corr_t",
                                    tag="corr")
                nc.scalar.dma_start(corr_t[:], corr_d.ap())

            # bulkier constants: tiles declared now, DMAs emitted mid-phase-A
            # (the framework coalesces DMA waits into a cumulative counter, so
            # anything emitted before the first transpose delays it)
            wv_t = cpool.tile([BDK, BDK + 1], bf16, name="wv_t", tag="wv")
            ia_t = cpool.tile([BDK + 1, BDK], f32, name="ia_t", tag="ia")
            sel_t = cpool.tile([BDK, U * BDK], bf16, name="sel_t", tag="sel")
            selt_t = cpool.tile([BDK, U * BDK], bf16, name="selt_t",
                                tag="selt")
            mkb_t = cpool.tile([BDK, BDK], bf16, name="mkb_t", tag="mkb")

            def emit_const_dmas():
                nc.sync.dma_start(wv_t[:], wv_d.ap())
                nc.scalar.dma_start(ia_t[:], ia_d.ap())
                nc.sync.dma_start(mkb_t[:], mkb_d.ap())
                nc.scalar.dma_start(sel_t[:], sel_d.ap())
                nc.sync.dma_start(selt_t[:], selt_d.ap())

            exp_all = cpool.tile([BDK, K * S], bf16, name="exp_all", tag="exp")
            inv_t = cpool.tile([BDK, K * T], bf16, name="inv_t", tag="inv")

            # --- phase A: transpose -> gram -> scores^T -> exp, skewed ---
            # PSUM tags (8 banks): big=tp/o_ps x4, gsb=g/seg/b x2, tiny x2
            TC = BDK + 1         # 120: chunk col stride (4B-aligned in PSUM)
            state = {"ncopy": 0, "loaded": 1, "a_left": S, "d_after_a": 0,
                     "interleave": False, "recip_done": set(),
                     "attn_done": set()}
            g_tiles = {}

            # engine split: PSUM-reading elementwise work can only run on
            # DVE/ACT (GpSimd has no PSUM access). During A/D overlap, keep
            # A's copies on ACT and D's relus mostly on DVE so the in-order
            # engine queues don't cross-block; use both engines otherwise.
            def emit_transposes(s, gi):
                xt_ = x_tiles[s]
                tp = pp.tile([128, 8 * TC], bf16, name="tp", tag="tp",
                             bufs=3)
                for c8 in range(8):
                    col = G_CH * gi + 128 * c8
                    nc.tensor.transpose(
                        tp[:, TC * c8:TC * c8 + BDK],
                        xt_[0:BDK, col:col + 128], id_t[:],
                    )
                xt = wpool.tile([128, 8 * TC], bf16, name="xt", tag="xt",
                                bufs=4)
                if state["interleave"] or state["ncopy"] % 2 == 1:
                    nc.scalar.copy(xt[:], tp[:])
                else:
                    nc.vector.tensor_copy(xt[:], tp[:])
                state["ncopy"] += 1
                return xt

            def emit_gram(s, gi, xt):
                if s not in g_tiles:
                    g_tiles[s] = pp.tile([BDK + 1, BDK], f32, name=f"g{s}",
                                         tag="gsb", bufs=2)
                g_ps = g_tiles[s]
                for c8 in range(8):
                    nc.tensor.matmul(
                        g_ps[0:BDK, :], xt[:, TC * c8:TC * c8 + BDK],
                        xt[:, TC * c8:TC * c8 + BDK],
                        start=(gi == 0 and c8 == 0),
                        stop=(gi == n_grp - 1 and c8 == 7),
                    )
                if gi == n_grp - 1:
                    emit_tiny_chain(s)

            def emit_tiny_chain(s):
                g_sb = wpool.tile([BDK, BDK], f32, name="g_sb", tag="g_sb",
                                  bufs=2)
                nc.vector.tensor_mul(g_sb[:], g_tiles[s][0:BDK, :], mk_t[:])
                m1_ps = pp.tile([BDK, K], f32, name="m1", tag="tiny", bufs=1)
                nc.tensor.matmul(m1_ps[:], g_sb[:], wq_t[:], start=True,
                                 stop=True)
                m1_sb = wpool.tile([BDK, K], f32, name="m1_sb", tag="m1_sb",
                                   bufs=2)
                nc.scalar.copy(m1_sb[:], m1_ps[:])
                st_ps = pp.tile([BDK, K], f32, name="st", tag="tiny", bufs=1)
                nc.tensor.matmul(st_ps[:], wk_t[:], m1_sb[:], start=True,
                                 stop=True)
                esl = slice(K * s, K * (s + 1))
                if have_bias:
                    eb_sb = wpool.tile([BDK, K], f32, name="eb_sb", tag="eb")
                    nc.vector.tensor_add(eb_sb[:], st_ps[:], corr_t[:, esl])
                    nc.scalar.activation(exp_all[:, esl], eb_sb[:], Exp)
                else:
                    nc.scalar.activation(exp_all[:, esl], st_ps[:], Exp)

            pend = []

            def gen_A(s):
                if s == 2:
                    emit_const_dmas()
                for gi in range(n_grp):
                    pend.append((s, gi, emit_transposes(s, gi)))
                    # deep prefetch: queue loads well ahead — ring FIFO means
                    # earlier loads still complete first, and a deep backlog
                    # keeps all 16 SDMA engines streaming at full rate
                    while state["loaded"] < min(3 * s + 2, S):
                        load_xtile(state["loaded"])
                        state["loaded"] += 1
                    if len(pend) > 2:
                        ps, pgi, xt = pend.pop(0)
                        emit_gram(ps, pgi, xt)
                    yield
                state["a_left"] -= 1

            def drain_pend():
                while pend:
                    ps, pgi, xt = pend.pop(0)
                    emit_gram(ps, pgi, xt)

            # --- per-tile segment sums -> reciprocal (sliding window) ---
            def emit_C_tile(t):
                idxs = [i for i, (ss, tt) in enumerate(used) if tt == t]
                seg_ps = pp.tile([BDK + 1, BDK], f32, name=f"seg{t}",
                                 tag="gsb", bufs=2)
                for n, i in enumerate(idxs):
                    s = used[i][0]
                    nc.tensor.matmul(
                        seg_ps[0:BDK, 0:K],
                        sel_t[:, BDK * i:BDK * (i + 1)],
                        exp_all[:, K * s:K * (s + 1)],
                        start=(n == 0), stop=(n == len(idxs) - 1),
                    )
                seg_sb = wpool.tile([BDK, K], f32, name="seg_sb",
                                    tag="seg_sb")
                nc.vector.tensor_scalar_max(
                    seg_sb[:], seg_ps[0:BDK, 0:K], 1e-30)
                nc.vector.reciprocal(inv_t[:, K * t:K * (t + 1)],
                                     seg_sb[:])
                state["recip_done"].add(t)

            # --- phase D: pipelined per stack ---
            state["nrelu"] = 0
            attn_tiles = {}

            def emit_attn_chain(s):
                state["attn_done"].add(s)
                idxs = [i for i, (ss, tt) in enumerate(used) if ss == s]
                invb_ps = pp.tile([BDK, K], f32, name="invb", tag="tiny",
                                  bufs=1)
                for n, i in enumerate(idxs):
                    t = used[i][1]
                    nc.tensor.matmul(
                        invb_ps[:],
                        selt_t[:, BDK * i:BDK * (i + 1)],
                        inv_t[:, K * t:K * (t + 1)],
                        start=(n == 0), stop=(n == len(idxs) - 1),
                    )
                attn_sb = wpool.tile([BDK, K], bf16, name="attn_sb",
                                     tag="attn_c", bufs=2)
                nc.vector.tensor_mul(attn_sb[:], exp_all[:, K * s:K * (s + 1)],
                                     invb_ps[:])
                # block-diag mask applied in one op via a 0-stride broadcast
                attn_bd = wpool.tile([BDK, BDK], bf16, name="attn_bd",
                                     tag="attn", bufs=2)
                attn_rep = attn_sb[:].unsqueeze(1).to_broadcast((BDK, BD, K))
                nc.vector.tensor_mul(attn_bd[:], attn_rep, mkb_t[:])
                attn_tiles[s] = attn_bd

            def gen_D(s):
                b_ps = pp.tile([BDK + 1, BDK], f32, name="b_ps", tag="gsb",
                               bufs=2)
                nc.tensor.matmul(b_ps[:], wv_t[:], attn_tiles.pop(s)[:],
                                 start=True, stop=True)
                # pad B to 128 weight cols: enables PE fast weight load on
                # the output matmuls (extra PSUM rows are never read)
                b_sb = wpool.tile([BDK + 1, WPAD], bf16, name="b_sb", tag="B",
                                  bufs=2)
                nc.vector.tensor_add(b_sb[:, 0:BDK], b_ps[:], ia_t[:])
                nc.gpsimd.memset(b_sb[:, BDK:WPAD], 0.0)
                nxt = s + 1
                if (nxt < S and nxt not in state["attn_done"]
                        and all(t in state["recip_done"]
                                for t in tiles_of[nxt])):
                    emit_attn_chain(nxt)  # overlaps this stack's matmuls

                osb = opool.tile([BDK, HW], bf16, name="osb", tag="osb",
                                 bufs=3)
                xt_ = x_tiles[s]
                tail = state["a_left"] == 0 and state["d_after_a"] >= 2
                if state["a_left"] == 0:
                    state["d_after_a"] += 1
                yield
                for oc in range(HW // O_CH):
                    sl = slice(O_CH * oc, O_CH * (oc + 1))
                    o_ps = pp.tile([WPAD, O_CH], f32, name="o_ps", tag="ops",
                                   bufs=2)
                    nc.tensor.matmul(o_ps[:], b_sb[:], xt_[:, sl],
                                     start=True, stop=True)
                    on_act = (oc % 2 == 1) if tail else (oc % 4 == 3)
                    if on_act:
                        nc.scalar.activation(osb[:, sl], o_ps[0:BDK, :], Relu)
                    else:
                        nc.vector.tensor_scalar_max(osb[:, sl],
                                                    o_ps[0:BDK, :], 0.0)
                    state["nrelu"] += 1
                    if oc % 2 == 1 and oc < 7:
                        yield
                # early stores ride the otherwise-empty sync HWDGE ring
                # (overlapping the SWDGE loads); late stores ride the SWDGE
                # ring once the loads have drained
                if s < S // 2:
                    nc.sync.dma_start(
                        y_d.ap()[:, s * HW:(s + 1) * HW], osb[:])
                else:
                    nc.gpsimd.dma_start(
                        y_d.ap()[:, s * HW:(s + 1) * HW], osb[:])

            # --- schedule: sliding window — emit tile t's recip one stack
            # after its last contribution (the exp has drained by then), and
            # emit D stacks trailing A by >=2 stacks, so the in-order engine
            # streams never hit a semaphore stall (stalls re-throttle the PE
            # clock via HAM). Loads and stores interleave on the ring. ---
            # Each D's output chunks are zipped BETWEEN the next A stack's
            # transpose groups so a relu-paced output chunk never stalls the
            # in-order PE queue.
            state["interleave"] = False
            d_next = 0
            dg = None
            dstate = {"dg": None}

            def start_D_if_ready(s):
                nonlocal d_next
                if (dstate["dg"] is None and d_next < S and d_next <= s - 2
                        and all(t in state["recip_done"]
                                for t in tiles_of[d_next])):
                    state["interleave"] = True
                    if d_next not in state["attn_done"]:
                        emit_attn_chain(d_next)
                    dstate["dg"] = gen_D(d_next)
                    next(dstate["dg"])
                    d_next += 1

            def step_D():
                if dstate["dg"] is not None:
                    try:
                        next(dstate["dg"])
                    except StopIteration:
                        dstate["dg"] = None

            for s in range(S):
                for _ in gen_A(s):
                    step_D()
                for t in range(T):
                    if ready[t] == s - 1:
                        emit_C_tile(t)
                start_D_if_ready(s)
            drain_pend()
            while dstate["dg"] is not None:
                step_D()
            for t in range(T):
                if t not in state["recip_done"]:
                    emit_C_tile(t)
            while d_next < S:
                if d_next not in state["attn_done"]:
                    emit_attn_chain(d_next)
                for _ in gen_D(d_next):
                    pass
                d_next += 1

    nc.compile()
    return nc


def _get_compiled(S, T, have_bias: bool, used: tuple):
    key = (S, T, have_bias, used)
    if key not in _cache:
        _cache[key] = _build(S, T, have_bias, used)
    return _cache[key]


def _bd7(m: np.ndarray) -> np.ndarray:
    out = np.zeros((BDK, BDK), dtype=np.float32)
    for j in range(BD):
        out[K * j:K * (j + 1), K * j:K * (j + 1)] = m
    return out


W_TILE = 14  # persons per tile window: smaller -> tighter A->D pipeline


def _plan(ids: np.ndarray):
    """Split persons into N_CORES contiguous chunks at imgid boundaries.
    All cores run the same compiled program padded to S stacks, so the
    objective is minimizing the max chunk (greedy furthest-reach at the
    minimal per-core stack budget)."""
    change = np.flatnonzero(np.diff(ids)) + 1
    allb = np.concatenate([[0], change, [P_TOTAL]]).astype(np.int64)

    Smin = math.ceil(P_TOTAL / (N_CORES * BD))
    for S in range(Smin, Smin + 4):
        bounds = [0]
        for _ in range(N_CORES):
            a = bounds[-1]
            cand = allb[(allb >= a) & (allb <= a + BD * S)]
            bounds.append(int(cand[-1]))
            if bounds[-1] == P_TOTAL:
                break
        while len(bounds) < N_CORES + 1:
            bounds.append(bounds[-1])
        if bounds[-1] == P_TOTAL:
            return bounds, S
    raise AssertionError("no feasible core split")


def _prepare(inputs: dict):
    import ml_dtypes
    nbf16 = ml_dtypes.bfloat16

    x = np.asarray(inputs["kpt_feat"], dtype=np.float32).reshape(
        P_TOTAL, K, HW)
    ids = np.asarray(inputs["imgid"]).astype(np.int64)
    Wq = np.asarray(inputs["Wq"], np.float32)
    Wk = np.asarray(inputs["Wk"], np.float32)
    Wv = np.asarray(inputs["Wv"], np.float32)
    bq = np.asarray(inputs["bq"], np.float32)
    bk = np.asarray(inputs["bk"], np.float32)
    bv = np.asarray(inputs["bv"], np.float32)

    bounds, S = _plan(ids)
    P_pad = S * BD
    # per-core local group index per person; each group goes to the tile
    # whose person-position window [t*W_TILE, (t+1)*W_TILE) contains its
    # first person (aligned across cores so tile-ready stacks line up),
    # spilling forward if a tile's 7 slot rows fill up
    lgs = []
    gslots = []  # per core: group -> (tile, slot row)
    T = 1
    for ci in range(N_CORES):
        a, b = bounds[ci], bounds[ci + 1]
        if b > a:
            _, starts, lg = np.unique(ids[a:b], return_index=True,
                                      return_inverse=True)
        else:
            starts = np.zeros((0,), np.int64)
            lg = np.zeros((0,), np.int64)
        cnt: dict = {}
        tprev = 0
        gs = []
        for p0 in starts:
            t = max(tprev, int(p0) // W_TILE)
            while cnt.get(t, 0) >= BD:
                t += 1
            gs.append((t, cnt.get(t, 0)))
            cnt[t] = cnt.get(t, 0) + 1
            tprev = t
        lgs.append(lg)
        gslots.append(gs)
    # compress tile ids to a dense global range (windows can be empty)
    present = sorted({t for gs in gslots for (t, _) in gs})
    remap = {t: i for i, t in enumerate(present)}
    gslots = [[(remap[t], r) for (t, r) in gs] for gs in gslots]
    T = max(1, len(present))

    wq_col = np.zeros((BDK, K), np.float32)
    for j in range(BD):
        wq_col[K * j:K * (j + 1), :] = Wq.T / NORM
    wkt_bd = _bd7(Wk.T.astype(np.float32))
    wv_aug = np.zeros((BDK, BDK + 1), np.float32)
    wv_aug[:, :BDK] = _bd7(Wv)
    for j in range(BD):
        wv_aug[K * j:K * (j + 1), BDK] = bv
    wv_aug = wv_aug.astype(nbf16)
    id119 = np.eye(BDK, dtype=np.float32).astype(nbf16)
    iaug = np.zeros((BDK + 1, BDK), np.float32)
    iaug[:BDK, :BDK] = np.eye(BDK, dtype=np.float32)
    bdmask = _bd7(np.ones((K, K), np.float32))

    have_bias = bool(np.any(bq) or np.any(bk))
    if have_bias:
        xsum = x.sum(axis=2)
        qx = xsum @ Wq.T
        kx = xsum @ Wk.T
        corr_all = (bk[None, :, None] * qx[:, None, :]
                    + bq[None, None, :] * kx[:, :, None]
                    + HW * (bq[None, None, :] * bk[None, :, None])) / NORM
        corr_all = corr_all.astype(np.float32)  # [P, m, i]
    else:
        corr_all = None

    xb = x.astype(nbf16)

    # selector tensors per core: group g -> slot row (g % GPT) of tile
    # (g // GPT); padding persons have all-zero selector rows (their exp
    # contributes nowhere and their attn comes out zero)
    eye = np.eye(K, dtype=np.float32)
    sels = []
    newpos_all = []
    used_set = set()
    for ci in range(N_CORES):
        a, b = bounds[ci], bounds[ci + 1]
        pc = b - a
        newpos_all.append(np.arange(pc))
        lg = lgs[ci]
        gs = gslots[ci]
        sel = np.zeros((S, T, BDK, BDK), np.float32)
        for pos in range(pc):
            s, j = divmod(pos, BD)
            t, lgi = gs[int(lg[pos])]
            sel[s, t, K * j:K * (j + 1), K * lgi:K * (lgi + 1)] = eye
            used_set.add((s, t))
        sels.append(sel)
    used = tuple(sorted(used_set))

    in_maps = []
    for ci in range(N_CORES):
        a, b = bounds[ci], bounds[ci + 1]
        pc = b - a
        # partition-major x: [120, S*HW]; row 119 = ones (residual fold)
        np_ = newpos_all[ci]
        rows = np.zeros((P_pad, K, HW), dtype=nbf16)
        if pc:
            rows[np_] = xb[a:b]
        rows = rows.reshape(P_pad * K, HW)
        arr3 = np.zeros((S, BDK + 1, HW), dtype=nbf16)
        arr3[:, :BDK] = rows.reshape(S, BDK, HW)
        arr3[:, BDK] = 1.0
        xs = np.ascontiguousarray(
            arr3.transpose(1, 0, 2).reshape(BDK + 1, S * HW))
        sel = sels[ci]
        su = np.stack([sel[s, t] for (s, t) in used])  # [U, 119, 119]
        sel_pack = su.transpose(1, 0, 2).reshape(BDK, len(used) * BDK)
        selt_pack = su.transpose(2, 0, 1).reshape(BDK, len(used) * BDK)
        m = {
            "x": xs,
            "wq_col": wq_col,
            "wkt_bd": wkt_bd,
            "wv_aug": wv_aug,
            "id119": id119,
            "iaug": iaug,
            "bdmask": bdmask,
            "bdmaskb": bdmask.astype(nbf16),
            "sel": np.ascontiguousarray(sel_pack).astype(nbf16),
            "selT": np.ascontiguousarray(selt_pack).astype(nbf16),
        }
        if have_bias:
            corr_col = np.zeros((BDK, K * S), np.float32)
            if pc:
                cpad = np.zeros((P_pad, K, K), np.float32)
                cpad[np_] = corr_all[a:b]
                for s in range(S):
                    for j in range(BD):
                        corr_col[K * j:K * (j + 1), K * s:K * (s + 1)] = \
                            cpad[BD * s + j]
            m["corr_col"] = corr_col
        in_maps.append(m)
    return in_maps, bounds, newpos_all, (S, T), have_bias, used


def _gather(results, bounds, newpos_all, S):
    out = np.empty((P_TOTAL, K, 64, 64), dtype=np.float32)
    for ci in range(N_CORES):
        a, b = bounds[ci], bounds[ci + 1]
        pc = b - a
        if pc:
            y = np.asarray(results[ci]["y"], dtype=np.float32)  # [119, S*HW]
            y = y.reshape(BDK, S, HW).transpose(1, 0, 2).reshape(
                S * BD, K, 64, 64)
            out[a:b] = y[newpos_all[ci]]
    return out


def _run(inputs: dict, trace: bool = False):
    _ensure_path()
    from concourse.bass_utils import run_bass_kernel_spmd

    in_maps, bounds, newpos_all, (S, T), have_bias, used = \
        _prepare(inputs)
    nc = _get_compiled(S, T, have_bias, used)
    res = run_bass_kernel_spmd(nc, in_maps, list(range(N_CORES)), trace=trace)
    return _gather(res.results, bounds, newpos_all, S), res


def kernel(**inputs) -> np.ndarray:
    out, _ = _run(inputs, trace=False)
    return out


# revision 59
# speedup vs baseline: 1.0841x; 1.0841x over previous
"""Trainium2 Bass kernel for nn_JointRelationModule (self-contained).

Math (per person p; softmax is segment-softmax over persons within an imgid
group, elementwise over the (K,K) score entries):
    q = Wq x + bq ; k = Wk x + bk ; v = Wv x + bv      (1x1 conv over K=17)
    S_p = q_p k_p^T / 64
    attn = segment-softmax over persons
    out = relu(attn_p @ v_p + x_p)

Device formulation (heavy ops bf16 on the PE, block-column layouts):
  - Stack BD=7 persons as [119, hw]. Per stack: G = x x^T via PE transpose +
    accumulating matmuls (bf16, f32 PSUM).
  - scores^T in block-column layout [119, 17] via a masked-Gram matmul chain
    (block-diag mask kills cross-person terms), so no gather/scatter DMAs.
  - Segment softmax via per-stack selector matmuls into group-slot tiles,
    reciprocal, selector-transpose broadcast back; all partition-aligned.
  - Output: B = blockdiag((attn Wv)^T) + I with an av row appended; the
    residual and v-bias ride along x_aug (all-ones row), so each output chunk
    is one matmul + one relu. B is zero-padded to 128 weight columns so the
    PE fast-weight-load path kicks in. Stored bf16, host upcasts.

Data movement: x and y live in a partition-major layout [120, S*hw]. All bulk
x loads / y stores ride the gpsimd (SWDGE) ring: its descriptors spread
evenly over all 16 SDMA engines, unlike the HWDGE (sync/scalar) rings which
concentrate on engines 0-6. Small constants ride the otherwise-idle HWDGE
rings. Stack 0's load is column-chunked so the first transposes start early.

Sharding: data-parallel over persons at imgid group boundaries (8 cores),
weights replicated. Host casts x to bf16 (halves load bytes); output comes
back bf16 (halves store bytes). Tolerance 2e-2; measured error ~5e-3.
"""

import math
import sys

import numpy as np

K = 17
HW = 4096  # 64*64
P_TOTAL = 512
N_CORES = 8
NORM = 64.0
BD = 7          # persons per stack
BDK = BD * K    # 119
O_CH = 512      # output chunk cols (one PSUM bank of f32)
WPAD = 128      # output-matmul weight cols padded for fast weight load

_cache: dict = {}


def _ensure_path():
    try:
        import concourse.bass  # noqa: F401
    except ImportError:
        for p in ("/opt/trn_rl_repo", "/root/.axon_site/_ro/trn_rl_repo"):
            if p not in sys.path:
                sys.path.insert(0, p)
        import concourse.bass  # noqa: F401


def _build(S: int, T: int, have_bias: bool, used: tuple):
    """Builds + compiles the per-core SPMD Bass program.

    Sliding-window softmax: group-slot tiles hold only a few groups each, so
    tile t's reciprocal is ready as soon as its last contributing stack's exp
    is done; output stacks trail input stacks by the tile span (~3 stacks).
    Loads and stores interleave continuously on the SWDGE ring."""
    _ensure_path()
    import concourse.bacc as bacc
    import concourse.mybir as mybir
    import concourse.tile as tile

    f32 = mybir.dt.float32
    bf16 = mybir.dt.bfloat16
    Exp = mybir.ActivationFunctionType.Exp
    Relu = mybir.ActivationFunctionType.Relu

    U = len(used)
    tiles_of = {s: sorted({t for (ss, t) in used if ss == s})
                for s in range(S)}
    ready = {t: max(ss for (ss, tt) in used if tt == t)
             for t in range(T)}

    nc = bacc.Bacc(
        "TRN2",
        target_bir_lowering=False,
        debug=False,
        enable_asserts=False,
        num_devices=N_CORES,
    )

    x_d = nc.dram_tensor("x", [BDK + 1, S * HW], bf16, kind="ExternalInput")
    wq_d = nc.dram_tensor("wq_col", [BDK, K], f32, kind="ExternalInput")
    wk_d = nc.dram_tensor("wkt_bd", [BDK, BDK], f32, kind="ExternalInput")
    wv_d = nc.dram_tensor("wv_aug", [BDK, BDK + 1], bf16, kind="ExternalInput")
    id_d = nc.dram_tensor("id119", [BDK, BDK], bf16, kind="ExternalInput")
    ia_d = nc.dram_tensor("iaug", [BDK + 1, BDK], f32, kind="ExternalInput")
    mk_d = nc.dram_tensor("bdmask", [BDK, BDK], f32, kind="ExternalInput")
    mkb_d = nc.dram_tensor("bdmaskb", [BDK, BDK], bf16, kind="ExternalInput")
    sel_d = nc.dram_tensor("sel", [BDK, U * BDK], bf16, kind="ExternalInput")
    selt_d = nc.dram_tensor("selT", [BDK, U * BDK], bf16,
                            kind="ExternalInput")
    if have_bias:
        corr_d = nc.dram_tensor("corr_col", [BDK, K * S], f32,
                                kind="ExternalInput")
    y_d = nc.dram_tensor("y", [BDK, S * HW], bf16, kind="ExternalOutput")

    G_CH = 1024          # x cols per transpose group
    n_grp = HW // G_CH   # 4 groups per stack

    with tile.TileContext(nc) as tc:
        with (
            nc.allow_low_precision(reason="bf16 softmax ok at 2e-2 tol"),
            tc.tile_pool(name="xpool", bufs=1) as xpool,
            tc.tile_pool(name="cpool", bufs=1) as cpool,
            tc.tile_pool(name="wpool", bufs=2) as wpool,
            tc.tile_pool(name="opool", bufs=2) as opool,
            tc.tile_pool(name="pp", bufs=2, space="PSUM") as pp,
        ):
            # --- tiny phase-A constants first (id_t gates every transpose);
            # consts ride the HWDGE rings, bulk x/y rides the SWDGE ring ---
            id_t = cpool.tile([BDK, BDK], bf16, name="id_t", tag="id")
            mk_t = cpool.tile([BDK, BDK], f32, name="mk_t", tag="mk")
            wq_t = cpool.tile([BDK, K], f32, name="wq_t", tag="wq")
            wk_t = cpool.tile([BDK, BDK], f32, name="wk_t", tag="wk")
            nc.sync.dma_start(id_t[:], id_d.ap())

            # HAM warm-up: ~2.5us of dummy matmuls on id_t while waiting for
            # the first x chunk, so the PE clock is at 2.4GHz (not the cold
            # 1.2GHz) when the real transposes start; the result is unread
            warm_ps = pp.tile([BDK, BDK], f32, name="warm", tag="tiny",
                              bufs=1)
            for _ in range(24):
                nc.tensor.matmul(warm_ps[:], id_t[:], id_t[:],
                                 start=True, stop=True)

            x_tiles = []  # per stack

            def load_xtile(s):
                xt_ = xpool.tile([BDK + 1, HW], bf16, name=f"xp{s}",
                                 tag=f"xp{s}")
                base = s * HW
                # chunked so transposes can start on a partial tile: the
                # first chunk's arrival, not the whole tile's, gates compute
                nch = 4 if s <= 2 else 2
                cw = HW // nch
                for ci_ in range(nch):
                    nc.gpsimd.dma_start(
                        xt_[:, cw * ci_:cw * (ci_ + 1)],
                        x_d.ap()[:, base + cw * ci_:base + cw * (ci_ + 1)])
                x_tiles.append(xt_)

            nc.scalar.dma_start(mk_t[:], mk_d.ap())
            nc.sync.dma_start(wq_t[:], wq_d.ap())
            nc.scalar.dma_start(wk_t[:], wk_d.ap())
            load_xtile(0)
            if have_bias:
                corr_t = cpool.tile([BDK, K * S], f32, name="corr_t",
                                    tag="corr")
                nc.scalar.dma_start(corr_t[:], corr_d.ap())

            # bulkier constants: tiles declared now, DMAs emitted mid-phase-A
            # (the framework coalesces DMA waits into a cumulative counter, so
            # anything emitted before the first transpose delays it)
            wv_t = cpool.tile([BDK, BDK + 1], bf16, name="wv_t", tag="wv")
            ia_t = cpool.tile([BDK + 1, BDK], f32, name="ia_t", tag="ia")
            sel_t = cpool.tile([BDK, U * BDK], bf16, name="sel_t", tag="sel")
            selt_t = cpool.tile([BDK, U * BDK], bf16, name="selt_t",
                                tag="selt")
            mkb_t = cpool.tile([BDK, BDK], bf16, name="mkb_t", tag="mkb")

            def emit_const_dmas():
                nc.sync.dma_start(wv_t[:], wv_d.ap())
                nc.scalar.dma_start(ia_t[:], ia_d.ap())
                nc.sync.dma_start(mkb_t[:], mkb_d.ap())
                nc.scalar.dma_start(sel_t[:], sel_d.ap())
                nc.sync.dma_start(selt_t[:], selt_d.ap())

            exp_all = cpool.tile([BDK, K * S], bf16, name="exp_all", tag="exp")
            inv_t = cpool.tile([BDK, K * T], bf16, name="inv_t", tag="inv")

            # --- phase A: transpose -> gram -> scores^T -> exp, skewed ---
            # PSUM tags (8 banks): big=tp/o_ps x4, gsb=g/seg/b x2, tiny x2
            TC = BDK + 1         # 120: chunk col stride (4B-aligned in PSUM)
            state = {"ncopy": 0, "loaded": 1, "a_left": S, "d_after_a": 0,
                     "interleave": False, "recip_done": set(),
                     "attn_done": set()}
            g_tiles = {}

            # engine split: PSUM-reading elementwise work can only run on
            # DVE/ACT (GpSimd has no PSUM access). During A/D overlap, keep
            # A's copies on ACT and D's relus mostly on DVE so the in-order
            # engine queues don't cross-block; use both engines otherwise.
            def emit_transposes(s, gi):
                xt_ = x_tiles[s]
                tp = pp.tile([128, 8 * TC], bf16, name="tp", tag="tp",
                             bufs=3)
                for c8 in range(8):
                    col = G_CH * gi + 128 * c8
                    nc.tensor.transpose(
                        tp[:, TC * c8:TC * c8 + BDK],
                        xt_[0:BDK, col:col + 128], id_t[:],
                    )
                xt = wpool.tile([128, 8 * TC], bf16, name="xt", tag="xt",
                                bufs=4)
                if state["interleave"] or state["ncopy"] % 2 == 1:
                    nc.scalar.copy(xt[:], tp[:])
                else:
                    nc.vector.tensor_copy(xt[:], tp[:])
                state["ncopy"] += 1
                return xt

            def emit_gram(s, gi, xt):
                if s not in g_tiles:
                    g_tiles[s] = pp.tile([BDK + 1, BDK], f32, name=f"g{s}",
                                         tag="gsb", bufs=2)
                g_ps = g_tiles[s]
                for c8 in range(8):
                    nc.tensor.matmul(
                        g_ps[0:BDK, :], xt[:, TC * c8:TC * c8 + BDK],
                        xt[:, TC * c8:TC * c8 + BDK],
                        start=(gi == 0 and c8 == 0),
                        stop=(gi == n_grp - 1 and c8 == 7),
                    )
                if gi == n_grp - 1:
                    emit_tiny_chain(s)

            def emit_tiny_chain(s):
                g_sb = wpool.tile([BDK, BDK], f32, name="g_sb", tag="g_sb",
                                  bufs=2)
                nc.vector.tensor_mul(g_sb[:], g_tiles[s][0:BDK, :], mk_t[:])
                m1_ps = pp.tile([BDK, K], f32, name="m1", tag="tiny", bufs=1)
                nc.tensor.matmul(m1_ps[:], g_sb[:], wq_t[:], start=True,
                                 stop=True)
                m1_sb = wpool.tile([BDK, K], f32, name="m1_sb", tag="m1_sb",
                                   bufs=2)
                nc.scalar.copy(m1_sb[:], m1_ps[:])
                st_ps = pp.tile([BDK, K], f32, name="st", tag="tiny", bufs=1)
                nc.tensor.matmul(st_ps[:], wk_t[:], m1_sb[:], start=True,
                                 stop=True)
                esl = slice(K * s, K * (s + 1))
                if have_bias:
                    eb_sb = wpool.tile([BDK, K], f32, name="eb_sb", tag="eb")
                    nc.vector.tensor_add(eb_sb[:], st_ps[:], corr_t[:, esl])
                    nc.scalar.activation(exp_all[:, esl], eb_sb[:], Exp)
                else:
                    nc.scalar.activation(exp_all[:, esl], st_ps[:], Exp)

            pend = []

            def gen_A(s):
                if s == 2:
                    emit_const_dmas()
                for gi in range(n_grp):
                    pend.append((s, gi, emit_transposes(s, gi)))
                    # just-in-time prefetch, interleaved with compute emission
                    while state["loaded"] < min(s + 4, S):
                        load_xtile(state["loaded"])
                        state["loaded"] += 1
                    if len(pend) > 2:
                        ps, pgi, xt = pend.pop(0)
                        emit_gram(ps, pgi, xt)
                    yield
                state["a_left"] -= 1

            def drain_pend():
                while pend:
                    ps, pgi, xt = pend.pop(0)
                    emit_gram(ps, pgi, xt)

            # --- per-tile segment sums -> reciprocal (sliding window) ---
            def emit_C_tile(t):
                idxs = [i for i, (ss, tt) in enumerate(used) if tt == t]
                seg_ps = pp.tile([BDK + 1, BDK], f32, name=f"seg{t}",
                                 tag="gsb", bufs=2)
                for n, i in enumerate(idxs):
                    s = used[i][0]
                    nc.tensor.matmul(
                        seg_ps[0:BDK, 0:K],
                        sel_t[:, BDK * i:BDK * (i + 1)],
                        exp_all[:, K * s:K * (s + 1)],
                        start=(n == 0), stop=(n == len(idxs) - 1),
                    )
                seg_sb = wpool.tile([BDK, K], f32, name="seg_sb",
                                    tag="seg_sb")
                nc.vector.tensor_scalar_max(
                    seg_sb[:], seg_ps[0:BDK, 0:K], 1e-30)
                nc.vector.reciprocal(inv_t[:, K * t:K * (t + 1)],
                                     seg_sb[:])
                state["recip_done"].add(t)

            # --- phase D: pipelined per stack ---
            state["nrelu"] = 0
            attn_tiles = {}

            def emit_attn_chain(s):
                state["attn_done"].add(s)
                idxs = [i for i, (ss, tt) in enumerate(used) if ss == s]
                invb_ps = pp.tile([BDK, K], f32, name="invb", tag="tiny",
                                  bufs=1)
                for n, i in enumerate(idxs):
                    t = used[i][1]
                    nc.tensor.matmul(
                        invb_ps[:],
                        selt_t[:, BDK * i:BDK * (i + 1)],
                        inv_t[:, K * t:K * (t + 1)],
                        start=(n == 0), stop=(n == len(idxs) - 1),
                    )
                attn_sb = wpool.tile([BDK, K], bf16, name="attn_sb",
                                     tag="attn_c", bufs=2)
                nc.vector.tensor_mul(attn_sb[:], exp_all[:, K * s:K * (s + 1)],
                                     invb_ps[:])
                # block-diag mask applied in one op via a 0-stride broadcast
                attn_bd = wpool.tile([BDK, BDK], bf16, name="attn_bd",
                                     tag="attn", bufs=2)
                attn_rep = attn_sb[:].unsqueeze(1).to_broadcast((BDK, BD, K))
                nc.vector.tensor_mul(attn_bd[:], attn_rep, mkb_t[:])
                attn_tiles[s] = attn_bd

            def gen_D(s):
                b_ps = pp.tile([BDK + 1, BDK], f32, name="b_ps", tag="gsb",
                               bufs=2)
                nc.tensor.matmul(b_ps[:], wv_t[:], attn_tiles.pop(s)[:],
                                 start=True, stop=True)
                # pad B to 128 weight cols: enables PE fast weight load on
                # the output matmuls (extra PSUM rows are never read)
                b_sb = wpool.tile([BDK + 1, WPAD], bf16, name="b_sb", tag="B",
                                  bufs=2)
                nc.vector.tensor_add(b_sb[:, 0:BDK], b_ps[:], ia_t[:])
                nc.gpsimd.memset(b_sb[:, BDK:WPAD], 0.0)
                nxt = s + 1
                if (nxt < S and nxt not in state["attn_done"]
                        and all(t in state["recip_done"]
                                for t in tiles_of[nxt])):
                    emit_attn_chain(nxt)  # overlaps this stack's matmuls

                osb = opool.tile([BDK, HW], bf16, name="osb", tag="osb",
                                 bufs=3)
                xt_ = x_tiles[s]
                tail = state["a_left"] == 0 and state["d_after_a"] >= 2
                if state["a_left"] == 0:
                    state["d_after_a"] += 1
                yield
                for oc in range(HW // O_CH):
                    sl = slice(O_CH * oc, O_CH * (oc + 1))
                    o_ps = pp.tile([WPAD, O_CH], f32, name="o_ps", tag="ops",
                                   bufs=2)
                    nc.tensor.matmul(o_ps[:], b_sb[:], xt_[:, sl],
                                     start=True, stop=True)
                    on_act = (oc % 2 == 1) if tail else (oc % 4 == 3)
                    if on_act:
                        nc.scalar.activation(osb[:, sl], o_ps[0:BDK, :], Relu)
                    else:
                        nc.vector.tensor_scalar_max(osb[:, sl],
                                                    o_ps[0:BDK, :], 0.0)
                    state["nrelu"] += 1
                    if oc == 3:
                        # first half of the store goes out as soon as its
                        # relus land: earlier store flow, shorter tail
                        seng = nc.sync if s < S // 2 else nc.gpsimd
                        hw2 = HW // 2
                        seng.dma_start(
                            y_d.ap()[:, s * HW:s * HW + hw2], osb[:, 0:hw2])
                    if oc % 2 == 1 and oc < 7:
                        yield
                # early stores ride the otherwise-empty sync HWDGE ring
                # (overlapping the SWDGE loads); late stores ride the SWDGE
                # ring once the loads have drained
                hw2 = HW // 2
                if s < S // 2:
                    nc.sync.dma_start(
                        y_d.ap()[:, s * HW + hw2:(s + 1) * HW],
                        osb[:, hw2:HW])
                else:
                    nc.gpsimd.dma_start(
                        y_d.ap()[:, s * HW + hw2:(s + 1) * HW],
                        osb[:, hw2:HW])

            # --- schedule: sliding window — emit tile t's recip one stack
            # after its last contribution (the exp has drained by then), and
            # emit D stacks trailing A by >=2 stacks, so the in-order engine
            # streams never hit a semaphore stall (stalls re-throttle the PE
            # clock via HAM). Loads and stores interleave on the ring. ---
            # Each D's output chunks are zipped BETWEEN the next A stack's
            # transpose groups so a relu-paced output chunk never stalls the
            # in-order PE queue.
            state["interleave"] = False
            d_next = 0
            dg = None
            dstate = {"dg": None}

            def start_D_if_ready(s):
                nonlocal d_next
                if (dstate["dg"] is None and d_next < S and d_next <= s - 2
                        and all(t in state["recip_done"]
                                for t in tiles_of[d_next])):
                    state["interleave"] = True
                    if d_next not in state["attn_done"]:
                        emit_attn_chain(d_next)
                    dstate["dg"] = gen_D(d_next)
                    next(dstate["dg"])
                    d_next += 1

            def step_D():
                if dstate["dg"] is not None:
                    try:
                        next(dstate["dg"])
                    except StopIteration:
                        dstate["dg"] = None

            for s in range(S):
                for _ in gen_A(s):
                    step_D()
                for t in range(T):
                    if ready[t] == s - 1:
                        emit_C_tile(t)
                start_D_if_ready(s)
            drain_pend()
            while dstate["dg"] is not None:
                step_D()
            for t in range(T):
                if t not in state["recip_done"]:
                    emit_C_tile(t)
            while d_next < S:
                if d_next not in state["attn_done"]:
                    emit_attn_chain(d_next)
                for _ in gen_D(d_next):
                    pass
                d_next += 1

    nc.compile()
    return nc


def _get_compiled(S, T, have_bias: bool, used: tuple):
    key = (S, T, have_bias, used)
    if key not in _cache:
        _cache[key] = _build(S, T, have_bias, used)
    return _cache[key]


def _bd7(m: np.ndarray) -> np.ndarray:
    out = np.zeros((BDK, BDK), dtype=np.float32)
    for j in range(BD):
        out[K * j:K * (j + 1), K * j:K * (j + 1)] = m
    return out


W_TILE = 14  # persons per tile window: smaller -> tighter A->D pipeline


def _plan(ids: np.ndarray):
    """Split persons into N_CORES contiguous chunks at imgid boundaries.
    All cores run the same compiled program padded to S stacks, so the
    objective is minimizing the max chunk (greedy furthest-reach at the
    minimal per-core stack budget)."""
    change = np.flatnonzero(np.diff(ids)) + 1
    allb = np.concatenate([[0], change, [P_TOTAL]]).astype(np.int64)

    Smin = math.ceil(P_TOTAL / (N_CORES * BD))
    for S in range(Smin, Smin + 4):
        bounds = [0]
        for _ in range(N_CORES):
            a = bounds[-1]
            cand = allb[(allb >= a) & (allb <= a + BD * S)]
            bounds.append(int(cand[-1]))
            if bounds[-1] == P_TOTAL:
                break
        while len(bounds) < N_CORES + 1:
            bounds.append(bounds[-1])
        if bounds[-1] == P_TOTAL:
            return bounds, S
    raise AssertionError("no feasible core split")


def _prepare(inputs: dict):
    import ml_dtypes
    nbf16 = ml_dtypes.bfloat16

    x = np.asarray(inputs["kpt_feat"], dtype=np.float32).reshape(
        P_TOTAL, K, HW)
    ids = np.asarray(inputs["imgid"]).astype(np.int64)
    Wq = np.asarray(inputs["Wq"], np.float32)
    Wk = np.asarray(inputs["Wk"], np.float32)
    Wv = np.asarray(inputs["Wv"], np.float32)
    bq = np.asarray(inputs["bq"], np.float32)
    bk = np.asarray(inputs["bk"], np.float32)
    bv = np.asarray(inputs["bv"], np.float32)

    bounds, S = _plan(ids)
    P_pad = S * BD
    # per-core local group index per person; each group goes to the tile
    # whose person-position window [t*W_TILE, (t+1)*W_TILE) contains its
    # first person (aligned across cores so tile-ready stacks line up),
    # spilling forward if a tile's 7 slot rows fill up
    lgs = []
    gslots = []  # per core: group -> (tile, slot row)
    T = 1
    for ci in range(N_CORES):
        a, b = bounds[ci], bounds[ci + 1]
        if b > a:
            _, starts, lg = np.unique(ids[a:b], return_index=True,
                                      return_inverse=True)
        else:
            starts = np.zeros((0,), np.int64)
            lg = np.zeros((0,), np.int64)
        cnt: dict = {}
        tprev = 0
        gs = []
        for p0 in starts:
            t = max(tprev, int(p0) // W_TILE)
            while cnt.get(t, 0) >= BD:
                t += 1
            gs.append((t, cnt.get(t, 0)))
            cnt[t] = cnt.get(t, 0) + 1
            tprev = t
        lgs.append(lg)
        gslots.append(gs)
    # compress tile ids to a dense global range (windows can be empty)
    present = sorted({t for gs in gslots for (t, _) in gs})
    remap = {t: i for i, t in enumerate(present)}
    gslots = [[(remap[t], r) for (t, r) in gs] for gs in gslots]
    T = max(1, len(present))

    wq_col = np.zeros((BDK, K), np.float32)
    for j in range(BD):
        wq_col[K * j:K * (j + 1), :] = Wq.T / NORM
    wkt_bd = _bd7(Wk.T.astype(np.float32))
    wv_aug = np.zeros((BDK, BDK + 1), np.float32)
    wv_aug[:, :BDK] = _bd7(Wv)
    for j in range(BD):
        wv_aug[K * j:K * (j + 1), BDK] = bv
    wv_aug = wv_aug.astype(nbf16)
    id119 = np.eye(BDK, dtype=np.float32).astype(nbf16)
    iaug = np.zeros((BDK + 1, BDK), np.float32)
    iaug[:BDK, :BDK] = np.eye(BDK, dtype=np.float32)
    bdmask = _bd7(np.ones((K, K), np.float32))

    have_bias = bool(np.any(bq) or np.any(bk))
    if have_bias:
        xsum = x.sum(axis=2)
        qx = xsum @ Wq.T
        kx = xsum @ Wk.T
        corr_all = (bk[None, :, None] * qx[:, None, :]
                    + bq[None, None, :] * kx[:, :, None]
                    + HW * (bq[None, None, :] * bk[None, :, None])) / NORM
        corr_all = corr_all.astype(np.float32)  # [P, m, i]
    else:
        corr_all = None

    xb = x.astype(nbf16)

    # selector tensors per core: group g -> slot row (g % GPT) of tile
    # (g // GPT); padding persons have all-zero selector rows (their exp
    # contributes nowhere and their attn comes out zero)
    eye = np.eye(K, dtype=np.float32)
    sels = []
    newpos_all = []
    used_set = set()
    for ci in range(N_CORES):
        a, b = bounds[ci], bounds[ci + 1]
        pc = b - a
        newpos_all.append(np.arange(pc))
        lg = lgs[ci]
        gs = gslots[ci]
        sel = np.zeros((S, T, BDK, BDK), np.float32)
        for pos in range(pc):
            s, j = divmod(pos, BD)
            t, lgi = gs[int(lg[pos])]
            sel[s, t, K * j:K * (j + 1), K * lgi:K * (lgi + 1)] = eye
            used_set.add((s, t))
        sels.append(sel)
    used = tuple(sorted(used_set))

    in_maps = []
    for ci in range(N_CORES):
        a, b = bounds[ci], bounds[ci + 1]
        pc = b - a
        # partition-major x: [120, S*HW]; row 119 = ones (residual fold)
        np_ = newpos_all[ci]
        rows = np.zeros((P_pad, K, HW), dtype=nbf16)
        if pc:
            rows[np_] = xb[a:b]
        rows = rows.reshape(P_pad * K, HW)
        arr3 = np.zeros((S, BDK + 1, HW), dtype=nbf16)
        arr3[:, :BDK] = rows.reshape(S, BDK, HW)
        arr3[:, BDK] = 1.0
        xs = np.ascontiguousarray(
            arr3.transpose(1, 0, 2).reshape(BDK + 1, S * HW))
        sel = sels[ci]
        su = np.stack([sel[s, t] for (s, t) in used])  # [U, 119, 119]
        sel_pack = su.transpose(1, 0, 2).reshape(BDK, len(used) * BDK)
        selt_pack = su.transpose(2, 0, 1).reshape(BDK, len(used) * BDK)
        m = {
            "x": xs,
            "wq_col": wq_col,
            "wkt_bd": wkt_bd,
            "wv_aug": wv_aug,
            "id119": id119,
            "iaug": iaug,
            "bdmask": bdmask,
            "bdmaskb": bdmask.astype(nbf16),
            "sel": np.ascontiguousarray(sel_pack).astype(nbf16),
            "selT": np.ascontiguousarray(selt_pack).astype(nbf16),
        }
        if have_bias:
            corr_col = np.zeros((BDK, K * S), np.float32)
            if pc:
                cpad = np.zeros((P_pad, K, K), np.float32)
                cpad[np_] = corr_all[a:b]
                for s in range(S):
                    for j in range(BD):
                        corr_col[K * j:K * (j + 1), K * s:K * (s + 1)] = \
                            cpad[BD * s + j]
            m["corr_col"] = corr_col
        in_maps.append(m)
    return in_maps, bounds, newpos_all, (S, T), have_bias, used


def _gather(results, bounds, newpos_all, S):
    out = np.empty((P_TOTAL, K, 64, 64), dtype=np.float32)
    for ci in range(N_CORES):
        a, b = bounds[ci], bounds[ci + 1]
        pc = b - a
        if pc:
            y = np.asarray(results[ci]["y"], dtype=np.float32)  # [119, S*HW]
            y = y.reshape(BDK, S, HW).transpose(1, 0, 2).reshape(
                S * BD, K, 64, 64)
            out[a:b] = y[newpos_all[ci]]
    return out


def _run(inputs: dict, trace: bool = False):
    _ensure_path()
    from concourse.bass_utils import run_bass_kernel_spmd

    in_maps, bounds, newpos_all, (S, T), have_bias, used = \
        _prepare(inputs)
    nc = _get_compiled(S, T, have_bias, used)
    res = run_bass_kernel_spmd(nc, in_maps, list(range(N_CORES)), trace=trace)
    return _gather(res.results, bounds, newpos_all, S), res


def kernel(**inputs) -> np.ndarray:
    out, _ = _run(inputs, trace=False)
    return out


# revision 60
# speedup vs baseline: 1.0954x; 1.0105x over previous
"""Trainium2 Bass kernel for nn_JointRelationModule (self-contained).

Math (per person p; softmax is segment-softmax over persons within an imgid
group, elementwise over the (K,K) score entries):
    q = Wq x + bq ; k = Wk x + bk ; v = Wv x + bv      (1x1 conv over K=17)
    S_p = q_p k_p^T / 64
    attn = segment-softmax over persons
    out = relu(attn_p @ v_p + x_p)

Device formulation (heavy ops bf16 on the PE, block-column layouts):
  - Stack BD=7 persons as [119, hw]. Per stack: G = x x^T via PE transpose +
    accumulating matmuls (bf16, f32 PSUM).
  - scores^T in block-column layout [119, 17] via a masked-Gram matmul chain
    (block-diag mask kills cross-person terms), so no gather/scatter DMAs.
  - Segment softmax via per-stack selector matmuls into group-slot tiles,
    reciprocal, selector-transpose broadcast back; all partition-aligned.
  - Output: B = blockdiag((attn Wv)^T) + I with an av row appended; the
    residual and v-bias ride along x_aug (all-ones row), so each output chunk
    is one matmul + one relu. B is zero-padded to 128 weight columns so the
    PE fast-weight-load path kicks in. Stored bf16, host upcasts.

Data movement: x and y live in a partition-major layout [120, S*hw]. All bulk
x loads / y stores ride the gpsimd (SWDGE) ring: its descriptors spread
evenly over all 16 SDMA engines, unlike the HWDGE (sync/scalar) rings which
concentrate on engines 0-6. Small constants ride the otherwise-idle HWDGE
rings. Stack 0's load is column-chunked so the first transposes start early.

Sharding: data-parallel over persons at imgid group boundaries (8 cores),
weights replicated. Host casts x to bf16 (halves load bytes); output comes
back bf16 (halves store bytes). Tolerance 2e-2; measured error ~5e-3.
"""

import math
import sys

import numpy as np

K = 17
HW = 4096  # 64*64
P_TOTAL = 512
N_CORES = 8
NORM = 64.0
BD = 7          # persons per stack
BDK = BD * K    # 119
O_CH = 512      # output chunk cols (one PSUM bank of f32)
WPAD = 128      # output-matmul weight cols padded for fast weight load

_cache: dict = {}


def _ensure_path():
    try:
        import concourse.bass  # noqa: F401
    except ImportError:
        for p in ("/opt/trn_rl_repo", "/root/.axon_site/_ro/trn_rl_repo"):
            if p not in sys.path:
                sys.path.insert(0, p)
        import concourse.bass  # noqa: F401


def _build(S: int, T: int, have_bias: bool, used: tuple):
    """Builds + compiles the per-core SPMD Bass program.

    Sliding-window softmax: group-slot tiles hold only a few groups each, so
    tile t's reciprocal is ready as soon as its last contributing stack's exp
    is done; output stacks trail input stacks by the tile span (~3 stacks).
    Loads and stores interleave continuously on the SWDGE ring."""
    _ensure_path()
    import concourse.bacc as bacc
    import concourse.mybir as mybir
    import concourse.tile as tile

    f32 = mybir.dt.float32
    bf16 = mybir.dt.bfloat16
    Exp = mybir.ActivationFunctionType.Exp
    Relu = mybir.ActivationFunctionType.Relu

    U = len(used)
    tiles_of = {s: sorted({t for (ss, t) in used if ss == s})
                for s in range(S)}
    ready = {t: max(ss for (ss, tt) in used if tt == t)
             for t in range(T)}

    nc = bacc.Bacc(
        "TRN2",
        target_bir_lowering=False,
        debug=False,
        enable_asserts=False,
        num_devices=N_CORES,
    )

    x_d = nc.dram_tensor("x", [BDK + 1, S * HW], bf16, kind="ExternalInput")
    wq_d = nc.dram_tensor("wq_col", [BDK, K], f32, kind="ExternalInput")
    wk_d = nc.dram_tensor("wkt_bd", [BDK, BDK], f32, kind="ExternalInput")
    wv_d = nc.dram_tensor("wv_aug", [BDK, BDK + 1], bf16, kind="ExternalInput")
    id_d = nc.dram_tensor("id119", [BDK, BDK], bf16, kind="ExternalInput")
    ia_d = nc.dram_tensor("iaug", [BDK + 1, BDK], f32, kind="ExternalInput")
    mk_d = nc.dram_tensor("bdmask", [BDK, BDK], f32, kind="ExternalInput")
    mkb_d = nc.dram_tensor("bdmaskb", [BDK, BDK], bf16, kind="ExternalInput")
    sel_d = nc.dram_tensor("sel", [BDK, U * BDK], bf16, kind="ExternalInput")
    selt_d = nc.dram_tensor("selT", [BDK, U * BDK], bf16,
                            kind="ExternalInput")
    if have_bias:
        corr_d = nc.dram_tensor("corr_col", [BDK, K * S], f32,
                                kind="ExternalInput")
    y_d = nc.dram_tensor("y", [BDK, S * HW], bf16, kind="ExternalOutput")

    G_CH = 1024          # x cols per transpose group
    n_grp = HW // G_CH   # 4 groups per stack

    with tile.TileContext(nc) as tc:
        with (
            nc.allow_low_precision(reason="bf16 softmax ok at 2e-2 tol"),
            tc.tile_pool(name="xpool", bufs=1) as xpool,
            tc.tile_pool(name="cpool", bufs=1) as cpool,
            tc.tile_pool(name="wpool", bufs=2) as wpool,
            tc.tile_pool(name="opool", bufs=2) as opool,
            tc.tile_pool(name="pp", bufs=2, space="PSUM") as pp,
        ):
            # --- tiny phase-A constants first (id_t gates every transpose);
            # consts ride the HWDGE rings, bulk x/y rides the SWDGE ring ---
            id_t = cpool.tile([BDK, BDK], bf16, name="id_t", tag="id")
            mk_t = cpool.tile([BDK, BDK], f32, name="mk_t", tag="mk")
            wq_t = cpool.tile([BDK, K], f32, name="wq_t", tag="wq")
            wk_t = cpool.tile([BDK, BDK], f32, name="wk_t", tag="wk")
            nc.sync.dma_start(id_t[:], id_d.ap())

            # HAM warm-up: ~2.5us of dummy matmuls on id_t while waiting for
            # the first x chunk, so the PE clock is at 2.4GHz (not the cold
            # 1.2GHz) when the real transposes start; the result is unread
            warm_ps = pp.tile([BDK, BDK], f32, name="warm", tag="tiny",
                              bufs=1)
            for _ in range(36):
                nc.tensor.matmul(warm_ps[:], id_t[:], id_t[:],
                                 start=True, stop=True)

            x_tiles = []  # per stack

            def load_xtile(s):
                xt_ = xpool.tile([BDK + 1, HW], bf16, name=f"xp{s}",
                                 tag=f"xp{s}")
                base = s * HW
                # chunked so transposes can start on a partial tile: the
                # first chunk's arrival, not the whole tile's, gates compute
                nch = 4
                cw = HW // nch
                for ci_ in range(nch):
                    nc.gpsimd.dma_start(
                        xt_[:, cw * ci_:cw * (ci_ + 1)],
                        x_d.ap()[:, base + cw * ci_:base + cw * (ci_ + 1)])
                x_tiles.append(xt_)

            nc.scalar.dma_start(mk_t[:], mk_d.ap())
            nc.sync.dma_start(wq_t[:], wq_d.ap())
            nc.scalar.dma_start(wk_t[:], wk_d.ap())
            load_xtile(0)
            if have_bias:
                corr_t = cpool.tile([BDK, K * S], f32, name="corr_t",
                                    tag="corr")
                nc.scalar.dma_start(corr_t[:], corr_d.ap())

            # bulkier constants: tiles declared now, DMAs emitted mid-phase-A
            # (the framework coalesces DMA waits into a cumulative counter, so
            # anything emitted before the first transpose delays it)
            wv_t = cpool.tile([BDK, BDK + 1], bf16, name="wv_t", tag="wv")
            ia_t = cpool.tile([BDK + 1, BDK], f32, name="ia_t", tag="ia")
            sel_t = cpool.tile([BDK, U * BDK], bf16, name="sel_t", tag="sel")
            selt_t = cpool.tile([BDK, U * BDK], bf16, name="selt_t",
                                tag="selt")
            mkb_t = cpool.tile([BDK, BDK], bf16, name="mkb_t", tag="mkb")

            def emit_const_dmas():
                nc.sync.dma_start(wv_t[:], wv_d.ap())
                nc.scalar.dma_start(ia_t[:], ia_d.ap())
                nc.sync.dma_start(mkb_t[:], mkb_d.ap())
                nc.scalar.dma_start(sel_t[:], sel_d.ap())
                nc.sync.dma_start(selt_t[:], selt_d.ap())

            exp_all = cpool.tile([BDK, K * S], bf16, name="exp_all", tag="exp")
            inv_t = cpool.tile([BDK, K * T], bf16, name="inv_t", tag="inv")

            # --- phase A: transpose -> gram -> scores^T -> exp, skewed ---
            # PSUM tags (8 banks): big=tp/o_ps x4, gsb=g/seg/b x2, tiny x2
            TC = BDK + 1         # 120: chunk col stride (4B-aligned in PSUM)
            state = {"ncopy": 0, "loaded": 1, "a_left": S, "d_after_a": 0,
                     "interleave": False, "recip_done": set(),
                     "attn_done": set()}
            g_tiles = {}

            # engine split: PSUM-reading elementwise work can only run on
            # DVE/ACT (GpSimd has no PSUM access). During A/D overlap, keep
            # A's copies on ACT and D's relus mostly on DVE so the in-order
            # engine queues don't cross-block; use both engines otherwise.
            def emit_transposes(s, gi):
                xt_ = x_tiles[s]
                tp = pp.tile([128, 8 * TC], bf16, name="tp", tag="tp",
                             bufs=3)
                for c8 in range(8):
                    col = G_CH * gi + 128 * c8
                    nc.tensor.transpose(
                        tp[:, TC * c8:TC * c8 + BDK],
                        xt_[0:BDK, col:col + 128], id_t[:],
                    )
                xt = wpool.tile([128, 8 * TC], bf16, name="xt", tag="xt",
                                bufs=4)
                if state["interleave"] or state["ncopy"] % 2 == 1:
                    nc.scalar.copy(xt[:], tp[:])
                else:
                    nc.vector.tensor_copy(xt[:], tp[:])
                state["ncopy"] += 1
                return xt

            def emit_gram(s, gi, xt):
                if s not in g_tiles:
                    g_tiles[s] = pp.tile([BDK + 1, BDK], f32, name=f"g{s}",
                                         tag="gsb", bufs=2)
                g_ps = g_tiles[s]
                for c8 in range(8):
                    nc.tensor.matmul(
                        g_ps[0:BDK, :], xt[:, TC * c8:TC * c8 + BDK],
                        xt[:, TC * c8:TC * c8 + BDK],
                        start=(gi == 0 and c8 == 0),
                        stop=(gi == n_grp - 1 and c8 == 7),
                    )
                if gi == n_grp - 1:
                    emit_tiny_chain(s)

            def emit_tiny_chain(s):
                g_sb = wpool.tile([BDK, BDK], f32, name="g_sb", tag="g_sb",
                                  bufs=2)
                nc.vector.tensor_mul(g_sb[:], g_tiles[s][0:BDK, :], mk_t[:])
                m1_ps = pp.tile([BDK, K], f32, name="m1", tag="tiny", bufs=1)
                nc.tensor.matmul(m1_ps[:], g_sb[:], wq_t[:], start=True,
                                 stop=True)
                m1_sb = wpool.tile([BDK, K], f32, name="m1_sb", tag="m1_sb",
                                   bufs=2)
                nc.scalar.copy(m1_sb[:], m1_ps[:])
                st_ps = pp.tile([BDK, K], f32, name="st", tag="tiny", bufs=1)
                nc.tensor.matmul(st_ps[:], wk_t[:], m1_sb[:], start=True,
                                 stop=True)
                esl = slice(K * s, K * (s + 1))
                if have_bias:
                    eb_sb = wpool.tile([BDK, K], f32, name="eb_sb", tag="eb")
                    nc.vector.tensor_add(eb_sb[:], st_ps[:], corr_t[:, esl])
                    nc.scalar.activation(exp_all[:, esl], eb_sb[:], Exp)
                else:
                    nc.scalar.activation(exp_all[:, esl], st_ps[:], Exp)

            pend = []

            def gen_A(s):
                if s == 2:
                    emit_const_dmas()
                for gi in range(n_grp):
                    pend.append((s, gi, emit_transposes(s, gi)))
                    # just-in-time prefetch, interleaved with compute emission
                    while state["loaded"] < min(s + 4, S):
                        load_xtile(state["loaded"])
                        state["loaded"] += 1
                    if len(pend) > 2:
                        ps, pgi, xt = pend.pop(0)
                        emit_gram(ps, pgi, xt)
                    yield
                state["a_left"] -= 1

            def drain_pend():
                while pend:
                    ps, pgi, xt = pend.pop(0)
                    emit_gram(ps, pgi, xt)

            # --- per-tile segment sums -> reciprocal (sliding window) ---
            def emit_C_tile(t):
                idxs = [i for i, (ss, tt) in enumerate(used) if tt == t]
                seg_ps = pp.tile([BDK + 1, BDK], f32, name=f"seg{t}",
                                 tag="gsb", bufs=2)
                for n, i in enumerate(idxs):
                    s = used[i][0]
                    nc.tensor.matmul(
                        seg_ps[0:BDK, 0:K],
                        sel_t[:, BDK * i:BDK * (i + 1)],
                        exp_all[:, K * s:K * (s + 1)],
                        start=(n == 0), stop=(n == len(idxs) - 1),
                    )
                seg_sb = wpool.tile([BDK, K], f32, name="seg_sb",
                                    tag="seg_sb")
                nc.vector.tensor_scalar_max(
                    seg_sb[:], seg_ps[0:BDK, 0:K], 1e-30)
                nc.vector.reciprocal(inv_t[:, K * t:K * (t + 1)],
                                     seg_sb[:])
                state["recip_done"].add(t)

            # --- phase D: pipelined per stack ---
            state["nrelu"] = 0
            attn_tiles = {}

            def emit_attn_chain(s):
                state["attn_done"].add(s)
                idxs = [i for i, (ss, tt) in enumerate(used) if ss == s]
                invb_ps = pp.tile([BDK, K], f32, name="invb", tag="tiny",
                                  bufs=1)
                for n, i in enumerate(idxs):
                    t = used[i][1]
                    nc.tensor.matmul(
                        invb_ps[:],
                        selt_t[:, BDK * i:BDK * (i + 1)],
                        inv_t[:, K * t:K * (t + 1)],
                        start=(n == 0), stop=(n == len(idxs) - 1),
                    )
                attn_sb = wpool.tile([BDK, K], bf16, name="attn_sb",
                                     tag="attn_c", bufs=2)
                nc.vector.tensor_mul(attn_sb[:], exp_all[:, K * s:K * (s + 1)],
                                     invb_ps[:])
                # block-diag mask applied in one op via a 0-stride broadcast
                attn_bd = wpool.tile([BDK, BDK], bf16, name="attn_bd",
                                     tag="attn", bufs=2)
                attn_rep = attn_sb[:].unsqueeze(1).to_broadcast((BDK, BD, K))
                nc.vector.tensor_mul(attn_bd[:], attn_rep, mkb_t[:])
                attn_tiles[s] = attn_bd

            def gen_D(s):
                b_ps = pp.tile([BDK + 1, BDK], f32, name="b_ps", tag="gsb",
                               bufs=2)
                nc.tensor.matmul(b_ps[:], wv_t[:], attn_tiles.pop(s)[:],
                                 start=True, stop=True)
                # pad B to 128 weight cols: enables PE fast weight load on
                # the output matmuls (extra PSUM rows are never read)
                b_sb = wpool.tile([BDK + 1, WPAD], bf16, name="b_sb", tag="B",
                                  bufs=2)
                nc.vector.tensor_add(b_sb[:, 0:BDK], b_ps[:], ia_t[:])
                nc.gpsimd.memset(b_sb[:, BDK:WPAD], 0.0)
                nxt = s + 1
                if (nxt < S and nxt not in state["attn_done"]
                        and all(t in state["recip_done"]
                                for t in tiles_of[nxt])):
                    emit_attn_chain(nxt)  # overlaps this stack's matmuls

                osb = opool.tile([BDK, HW], bf16, name="osb", tag="osb",
                                 bufs=3)
                xt_ = x_tiles[s]
                tail = state["a_left"] == 0 and state["d_after_a"] >= 2
                if state["a_left"] == 0:
                    state["d_after_a"] += 1
                yield
                for oc in range(HW // O_CH):
                    sl = slice(O_CH * oc, O_CH * (oc + 1))
                    o_ps = pp.tile([WPAD, O_CH], f32, name="o_ps", tag="ops",
                                   bufs=2)
                    nc.tensor.matmul(o_ps[:], b_sb[:], xt_[:, sl],
                                     start=True, stop=True)
                    on_act = (oc % 2 == 1) if tail else (oc % 4 == 3)
                    if on_act:
                        nc.scalar.activation(osb[:, sl], o_ps[0:BDK, :], Relu)
                    else:
                        nc.vector.tensor_scalar_max(osb[:, sl],
                                                    o_ps[0:BDK, :], 0.0)
                    state["nrelu"] += 1
                    if oc == 3:
                        # first half of the store goes out as soon as its
                        # relus land: earlier store flow, shorter tail
                        seng = nc.sync if s < S // 2 else nc.gpsimd
                        hw2 = HW // 2
                        seng.dma_start(
                            y_d.ap()[:, s * HW:s * HW + hw2], osb[:, 0:hw2])
                    if oc % 2 == 1 and oc < 7:
                        yield
                # early stores ride the otherwise-empty sync HWDGE ring
                # (overlapping the SWDGE loads); late stores ride the SWDGE
                # ring once the loads have drained
                hw2 = HW // 2
                if s < S // 2:
                    nc.sync.dma_start(
                        y_d.ap()[:, s * HW + hw2:(s + 1) * HW],
                        osb[:, hw2:HW])
                else:
                    nc.gpsimd.dma_start(
                        y_d.ap()[:, s * HW + hw2:(s + 1) * HW],
                        osb[:, hw2:HW])

            # --- schedule: sliding window — emit tile t's recip one stack
            # after its last contribution (the exp has drained by then), and
            # emit D stacks trailing A by >=2 stacks, so the in-order engine
            # streams never hit a semaphore stall (stalls re-throttle the PE
            # clock via HAM). Loads and stores interleave on the ring. ---
            # Each D's output chunks are zipped BETWEEN the next A stack's
            # transpose groups so a relu-paced output chunk never stalls the
            # in-order PE queue.
            state["interleave"] = False
            d_next = 0
            dg = None
            dstate = {"dg": None}

            def start_D_if_ready(s):
                nonlocal d_next
                if (dstate["dg"] is None and d_next < S and d_next <= s - 2
                        and all(t in state["recip_done"]
                                for t in tiles_of[d_next])):
                    state["interleave"] = True
                    if d_next not in state["attn_done"]:
                        emit_attn_chain(d_next)
                    dstate["dg"] = gen_D(d_next)
                    next(dstate["dg"])
                    d_next += 1

            def step_D():
                if dstate["dg"] is not None:
                    try:
                        next(dstate["dg"])
                    except StopIteration:
                        dstate["dg"] = None

            for s in range(S):
                for _ in gen_A(s):
                    step_D()
                for t in range(T):
                    if ready[t] == s - 1:
                        emit_C_tile(t)
                start_D_if_ready(s)
            drain_pend()
            while dstate["dg"] is not None:
                step_D()
            for t in range(T):
                if t not in state["recip_done"]:
                    emit_C_tile(t)
            while d_next < S:
                if d_next not in state["attn_done"]:
                    emit_attn_chain(d_next)
                for _ in gen_D(d_next):
                    pass
                d_next += 1

    nc.compile()
    return nc


def _get_compiled(S, T, have_bias: bool, used: tuple):
    key = (S, T, have_bias, used)
    if key not in _cache:
        _cache[key] = _build(S, T, have_bias, used)
    return _cache[key]


def _bd7(m: np.ndarray) -> np.ndarray:
    out = np.zeros((BDK, BDK), dtype=np.float32)
    for j in range(BD):
        out[K * j:K * (j + 1), K * j:K * (j + 1)] = m
    return out


W_TILE = 14  # persons per tile window: smaller -> tighter A->D pipeline


def _plan(ids: np.ndarray):
    """Split persons into N_CORES contiguous chunks at imgid boundaries.
    All cores run the same compiled program padded to S stacks, so the
    objective is minimizing the max chunk (greedy furthest-reach at the
    minimal per-core stack budget)."""
    change = np.flatnonzero(np.diff(ids)) + 1
    allb = np.concatenate([[0], change, [P_TOTAL]]).astype(np.int64)

    Smin = math.ceil(P_TOTAL / (N_CORES * BD))
    for S in range(Smin, Smin + 4):
        bounds = [0]
        for _ in range(N_CORES):
            a = bounds[-1]
            cand = allb[(allb >= a) & (allb <= a + BD * S)]
            bounds.append(int(cand[-1]))
            if bounds[-1] == P_TOTAL:
                break
        while len(bounds) < N_CORES + 1:
            bounds.append(bounds[-1])
        if bounds[-1] == P_TOTAL:
            return bounds, S
    raise AssertionError("no feasible core split")


def _prepare(inputs: dict):
    import ml_dtypes
    nbf16 = ml_dtypes.bfloat16

    x = np.asarray(inputs["kpt_feat"], dtype=np.float32).reshape(
        P_TOTAL, K, HW)
    ids = np.asarray(inputs["imgid"]).astype(np.int64)
    Wq = np.asarray(inputs["Wq"], np.float32)
    Wk = np.asarray(inputs["Wk"], np.float32)
    Wv = np.asarray(inputs["Wv"], np.float32)
    bq = np.asarray(inputs["bq"], np.float32)
    bk = np.asarray(inputs["bk"], np.float32)
    bv = np.asarray(inputs["bv"], np.float32)

    bounds, S = _plan(ids)
    P_pad = S * BD
    # per-core local group index per person; each group goes to the tile
    # whose person-position window [t*W_TILE, (t+1)*W_TILE) contains its
    # first person (aligned across cores so tile-ready stacks line up),
    # spilling forward if a tile's 7 slot rows fill up
    lgs = []
    gslots = []  # per core: group -> (tile, slot row)
    T = 1
    for ci in range(N_CORES):
        a, b = bounds[ci], bounds[ci + 1]
        if b > a:
            _, starts, lg = np.unique(ids[a:b], return_index=True,
                                      return_inverse=True)
        else:
            starts = np.zeros((0,), np.int64)
            lg = np.zeros((0,), np.int64)
        cnt: dict = {}
        tprev = 0
        gs = []
        for p0 in starts:
            t = max(tprev, int(p0) // W_TILE)
            while cnt.get(t, 0) >= BD:
                t += 1
            gs.append((t, cnt.get(t, 0)))
            cnt[t] = cnt.get(t, 0) + 1
            tprev = t
        lgs.append(lg)
        gslots.append(gs)
    # compress tile ids to a dense global range (windows can be empty)
    present = sorted({t for gs in gslots for (t, _) in gs})
    remap = {t: i for i, t in enumerate(present)}
    gslots = [[(remap[t], r) for (t, r) in gs] for gs in gslots]
    T = max(1, len(present))

    wq_col = np.zeros((BDK, K), np.float32)
    for j in range(BD):
        wq_col[K * j:K * (j + 1), :] = Wq.T / NORM
    wkt_bd = _bd7(Wk.T.astype(np.float32))
    wv_aug = np.zeros((BDK, BDK + 1), np.float32)
    wv_aug[:, :BDK] = _bd7(Wv)
    for j in range(BD):
        wv_aug[K * j:K * (j + 1), BDK] = bv
    wv_aug = wv_aug.astype(nbf16)
    id119 = np.eye(BDK, dtype=np.float32).astype(nbf16)
    iaug = np.zeros((BDK + 1, BDK), np.float32)
    iaug[:BDK, :BDK] = np.eye(BDK, dtype=np.float32)
    bdmask = _bd7(np.ones((K, K), np.float32))

    have_bias = bool(np.any(bq) or np.any(bk))
    if have_bias:
        xsum = x.sum(axis=2)
        qx = xsum @ Wq.T
        kx = xsum @ Wk.T
        corr_all = (bk[None, :, None] * qx[:, None, :]
                    + bq[None, None, :] * kx[:, :, None]
                    + HW * (bq[None, None, :] * bk[None, :, None])) / NORM
        corr_all = corr_all.astype(np.float32)  # [P, m, i]
    else:
        corr_all = None

    xb = x.astype(nbf16)

    # selector tensors per core: group g -> slot row (g % GPT) of tile
    # (g // GPT); padding persons have all-zero selector rows (their exp
    # contributes nowhere and their attn comes out zero)
    eye = np.eye(K, dtype=np.float32)
    sels = []
    newpos_all = []
    used_set = set()
    for ci in range(N_CORES):
        a, b = bounds[ci], bounds[ci + 1]
        pc = b - a
        newpos_all.append(np.arange(pc))
        lg = lgs[ci]
        gs = gslots[ci]
        sel = np.zeros((S, T, BDK, BDK), np.float32)
        for pos in range(pc):
            s, j = divmod(pos, BD)
            t, lgi = gs[int(lg[pos])]
            sel[s, t, K * j:K * (j + 1), K * lgi:K * (lgi + 1)] = eye
            used_set.add((s, t))
        sels.append(sel)
    used = tuple(sorted(used_set))

    in_maps = []
    for ci in range(N_CORES):
        a, b = bounds[ci], bounds[ci + 1]
        pc = b - a
        # partition-major x: [120, S*HW]; row 119 = ones (residual fold)
        np_ = newpos_all[ci]
        rows = np.zeros((P_pad, K, HW), dtype=nbf16)
        if pc:
            rows[np_] = xb[a:b]
        rows = rows.reshape(P_pad * K, HW)
        arr3 = np.zeros((S, BDK + 1, HW), dtype=nbf16)
        arr3[:, :BDK] = rows.reshape(S, BDK, HW)
        arr3[:, BDK] = 1.0
        xs = np.ascontiguousarray(
            arr3.transpose(1, 0, 2).reshape(BDK + 1, S * HW))
        sel = sels[ci]
        su = np.stack([sel[s, t] for (s, t) in used])  # [U, 119, 119]
        sel_pack = su.transpose(1, 0, 2).reshape(BDK, len(used) * BDK)
        selt_pack = su.transpose(2, 0, 1).reshape(BDK, len(used) * BDK)
        m = {
            "x": xs,
            "wq_col": wq_col,
            "wkt_bd": wkt_bd,
            "wv_aug": wv_aug,
            "id119": id119,
            "iaug": iaug,
            "bdmask": bdmask,
            "bdmaskb": bdmask.astype(nbf16),
            "sel": np.ascontiguousarray(sel_pack).astype(nbf16),
            "selT": np.ascontiguousarray(selt_pack).astype(nbf16),
        }
        if have_bias:
            corr_col = np.zeros((BDK, K * S), np.float32)
            if pc:
                cpad = np.zeros((P_pad, K, K), np.float32)
                cpad[np_] = corr_all[a:b]
                for s in range(S):
                    for j in range(BD):
                        corr_col[K * j:K * (j + 1), K * s:K * (s + 1)] = \
                            cpad[BD * s + j]
            m["corr_col"] = corr_col
        in_maps.append(m)
    return in_maps, bounds, newpos_all, (S, T), have_bias, used


def _gather(results, bounds, newpos_all, S):
    out = np.empty((P_TOTAL, K, 64, 64), dtype=np.float32)
    for ci in range(N_CORES):
        a, b = bounds[ci], bounds[ci + 1]
        pc = b - a
        if pc:
            y = np.asarray(results[ci]["y"], dtype=np.float32)  # [119, S*HW]
            y = y.reshape(BDK, S, HW).transpose(1, 0, 2).reshape(
                S * BD, K, 64, 64)
            out[a:b] = y[newpos_all[ci]]
    return out


def _run(inputs: dict, trace: bool = False):
    _ensure_path()
    from concourse.bass_utils import run_bass_kernel_spmd

    in_maps, bounds, newpos_all, (S, T), have_bias, used = \
        _prepare(inputs)
    nc = _get_compiled(S, T, have_bias, used)
    res = run_bass_kernel_spmd(nc, in_maps, list(range(N_CORES)), trace=trace)
    return _gather(res.results, bounds, newpos_all, S), res


def kernel(**inputs) -> np.ndarray:
    out, _ = _run(inputs, trace=False)
    return out


# revision 61
# speedup vs baseline: 1.1231x; 1.0253x over previous
"""Trainium2 Bass kernel for nn_JointRelationModule (self-contained).

Math (per person p; softmax is segment-softmax over persons within an imgid
group, elementwise over the (K,K) score entries):
    q = Wq x + bq ; k = Wk x + bk ; v = Wv x + bv      (1x1 conv over K=17)
    S_p = q_p k_p^T / 64
    attn = segment-softmax over persons
    out = relu(attn_p @ v_p + x_p)

Device formulation (heavy ops bf16 on the PE, block-column layouts):
  - Stack BD=7 persons as [119, hw]. Per stack: G = x x^T via PE transpose +
    accumulating matmuls (bf16, f32 PSUM).
  - scores^T in block-column layout [119, 17] via a masked-Gram matmul chain
    (block-diag mask kills cross-person terms), so no gather/scatter DMAs.
  - Segment softmax via per-stack selector matmuls into group-slot tiles,
    reciprocal, selector-transpose broadcast back; all partition-aligned.
  - Output: B = blockdiag((attn Wv)^T) + I with an av row appended; the
    residual and v-bias ride along x_aug (all-ones row), so each output chunk
    is one matmul + one relu. B is zero-padded to 128 weight columns so the
    PE fast-weight-load path kicks in. Stored bf16, host upcasts.

Data movement: x and y live in a partition-major layout [120, S*hw]. All bulk
x loads / y stores ride the gpsimd (SWDGE) ring: its descriptors spread
evenly over all 16 SDMA engines, unlike the HWDGE (sync/scalar) rings which
concentrate on engines 0-6. Small constants ride the otherwise-idle HWDGE
rings. Stack 0's load is column-chunked so the first transposes start early.

Sharding: data-parallel over persons at imgid group boundaries (8 cores),
weights replicated. Host casts x to bf16 (halves load bytes); output comes
back bf16 (halves store bytes). Tolerance 2e-2; measured error ~5e-3.
"""

import math
import sys

import numpy as np

K = 17
HW = 4096  # 64*64
P_TOTAL = 512
N_CORES = 8
NORM = 64.0
BD = 7          # persons per stack
BDK = BD * K    # 119
O_CH = 512      # output chunk cols (one PSUM bank of f32)
WPAD = 128      # output-matmul weight cols padded for fast weight load

_cache: dict = {}


def _ensure_path():
    try:
        import concourse.bass  # noqa: F401
    except ImportError:
        for p in ("/opt/trn_rl_repo", "/root/.axon_site/_ro/trn_rl_repo"):
            if p not in sys.path:
                sys.path.insert(0, p)
        import concourse.bass  # noqa: F401


def _build(S: int, T: int, have_bias: bool, used: tuple):
    """Builds + compiles the per-core SPMD Bass program.

    Sliding-window softmax: group-slot tiles hold only a few groups each, so
    tile t's reciprocal is ready as soon as its last contributing stack's exp
    is done; output stacks trail input stacks by the tile span (~3 stacks).
    Loads and stores interleave continuously on the SWDGE ring."""
    _ensure_path()
    import concourse.bacc as bacc
    import concourse.mybir as mybir
    import concourse.tile as tile

    f32 = mybir.dt.float32
    bf16 = mybir.dt.bfloat16
    Exp = mybir.ActivationFunctionType.Exp
    Relu = mybir.ActivationFunctionType.Relu

    U = len(used)
    tiles_of = {s: sorted({t for (ss, t) in used if ss == s})
                for s in range(S)}
    ready = {t: max(ss for (ss, tt) in used if tt == t)
             for t in range(T)}

    nc = bacc.Bacc(
        "TRN2",
        target_bir_lowering=False,
        debug=False,
        enable_asserts=False,
        num_devices=N_CORES,
    )

    x_d = nc.dram_tensor("x", [BDK + 1, S * HW], bf16, kind="ExternalInput")
    wq_d = nc.dram_tensor("wq_col", [BDK, K], f32, kind="ExternalInput")
    wk_d = nc.dram_tensor("wkt_bd", [BDK, BDK], f32, kind="ExternalInput")
    wv_d = nc.dram_tensor("wv_aug", [BDK, BDK + 1], bf16, kind="ExternalInput")
    id_d = nc.dram_tensor("id119", [BDK, BDK], bf16, kind="ExternalInput")
    ia_d = nc.dram_tensor("iaug", [BDK + 1, BDK], f32, kind="ExternalInput")
    mk_d = nc.dram_tensor("bdmask", [BDK, BDK], f32, kind="ExternalInput")
    mkb_d = nc.dram_tensor("bdmaskb", [BDK, BDK], bf16, kind="ExternalInput")
    sel_d = nc.dram_tensor("sel", [BDK, U * BDK], bf16, kind="ExternalInput")
    selt_d = nc.dram_tensor("selT", [BDK, U * BDK], bf16,
                            kind="ExternalInput")
    if have_bias:
        corr_d = nc.dram_tensor("corr_col", [BDK, K * S], f32,
                                kind="ExternalInput")
    y_d = nc.dram_tensor("y", [BDK, S * HW], bf16, kind="ExternalOutput")

    G_CH = 1024          # x cols per transpose group
    n_grp = HW // G_CH   # 4 groups per stack

    with tile.TileContext(nc) as tc:
        with (
            nc.allow_low_precision(reason="bf16 softmax ok at 2e-2 tol"),
            tc.tile_pool(name="xpool", bufs=1) as xpool,
            tc.tile_pool(name="cpool", bufs=1) as cpool,
            tc.tile_pool(name="wpool", bufs=2) as wpool,
            tc.tile_pool(name="opool", bufs=2) as opool,
            tc.tile_pool(name="pp", bufs=2, space="PSUM") as pp,
        ):
            # --- tiny phase-A constants first (id_t gates every transpose);
            # consts ride the HWDGE rings, bulk x/y rides the SWDGE ring ---
            id_t = cpool.tile([BDK, BDK], bf16, name="id_t", tag="id")
            mk_t = cpool.tile([BDK, BDK], f32, name="mk_t", tag="mk")
            wq_t = cpool.tile([BDK, K], f32, name="wq_t", tag="wq")
            wk_t = cpool.tile([BDK, BDK], f32, name="wk_t", tag="wk")
            nc.sync.dma_start(id_t[:], id_d.ap())

            # HAM warm-up: ~2.5us of dummy matmuls on id_t while waiting for
            # the first x chunk, so the PE clock is at 2.4GHz (not the cold
            # 1.2GHz) when the real transposes start; the result is unread
            warm_ps = pp.tile([BDK, BDK], f32, name="warm", tag="tiny",
                              bufs=1)
            for _ in range(36):
                nc.tensor.matmul(warm_ps[:], id_t[:], id_t[:],
                                 start=True, stop=True)

            x_tiles = []  # per stack

            def load_xtile(s):
                xt_ = xpool.tile([BDK + 1, HW], bf16, name=f"xp{s}",
                                 tag=f"xp{s}")
                base = s * HW
                # chunked so transposes can start on a partial tile: the
                # first chunk's arrival, not the whole tile's, gates compute
                nch = 4
                cw = HW // nch
                for ci_ in range(nch):
                    nc.gpsimd.dma_start(
                        xt_[:, cw * ci_:cw * (ci_ + 1)],
                        x_d.ap()[:, base + cw * ci_:base + cw * (ci_ + 1)])
                x_tiles.append(xt_)

            nc.scalar.dma_start(mk_t[:], mk_d.ap())
            nc.sync.dma_start(wq_t[:], wq_d.ap())
            nc.scalar.dma_start(wk_t[:], wk_d.ap())
            load_xtile(0)
            if have_bias:
                corr_t = cpool.tile([BDK, K * S], f32, name="corr_t",
                                    tag="corr")
                nc.scalar.dma_start(corr_t[:], corr_d.ap())

            # bulkier constants: tiles declared now, DMAs emitted mid-phase-A
            # (the framework coalesces DMA waits into a cumulative counter, so
            # anything emitted before the first transpose delays it)
            wv_t = cpool.tile([BDK, BDK + 1], bf16, name="wv_t", tag="wv")
            ia_t = cpool.tile([BDK + 1, BDK], f32, name="ia_t", tag="ia")
            sel_t = cpool.tile([BDK, U * BDK], bf16, name="sel_t", tag="sel")
            selt_t = cpool.tile([BDK, U * BDK], bf16, name="selt_t",
                                tag="selt")
            mkb_t = cpool.tile([BDK, BDK], bf16, name="mkb_t", tag="mkb")

            def emit_const_dmas():
                nc.sync.dma_start(wv_t[:], wv_d.ap())
                nc.scalar.dma_start(ia_t[:], ia_d.ap())
                nc.sync.dma_start(mkb_t[:], mkb_d.ap())
                nc.scalar.dma_start(sel_t[:], sel_d.ap())
                nc.sync.dma_start(selt_t[:], selt_d.ap())

            exp_all = cpool.tile([BDK, K * S], bf16, name="exp_all", tag="exp")
            inv_t = cpool.tile([BDK, K * T], bf16, name="inv_t", tag="inv")

            # --- phase A: transpose -> gram -> scores^T -> exp, skewed ---
            # PSUM tags (8 banks): big=tp/o_ps x4, gsb=g/seg/b x2, tiny x2
            TC = BDK + 1         # 120: chunk col stride (4B-aligned in PSUM)
            state = {"ncopy": 0, "loaded": 1, "a_left": S, "d_after_a": 0,
                     "interleave": False, "recip_done": set(),
                     "attn_done": set()}
            g_tiles = {}

            # engine split: PSUM-reading elementwise work can only run on
            # DVE/ACT (GpSimd has no PSUM access). During A/D overlap, keep
            # A's copies on ACT and D's relus mostly on DVE so the in-order
            # engine queues don't cross-block; use both engines otherwise.
            def emit_transposes(s, gi):
                xt_ = x_tiles[s]
                tp = pp.tile([128, 8 * TC], bf16, name="tp", tag="tp",
                             bufs=3)
                for c8 in range(8):
                    col = G_CH * gi + 128 * c8
                    nc.tensor.transpose(
                        tp[:, TC * c8:TC * c8 + BDK],
                        xt_[0:BDK, col:col + 128], id_t[:],
                    )
                xt = wpool.tile([128, 8 * TC], bf16, name="xt", tag="xt",
                                bufs=4)
                if state["interleave"] or state["ncopy"] % 2 == 1:
                    nc.scalar.copy(xt[:], tp[:])
                else:
                    nc.vector.tensor_copy(xt[:], tp[:])
                state["ncopy"] += 1
                return xt

            def emit_gram(s, gi, xt):
                if s not in g_tiles:
                    g_tiles[s] = pp.tile([BDK + 1, BDK], f32, name=f"g{s}",
                                         tag="gsb", bufs=2)
                g_ps = g_tiles[s]
                for c8 in range(8):
                    nc.tensor.matmul(
                        g_ps[0:BDK, :], xt[:, TC * c8:TC * c8 + BDK],
                        xt[:, TC * c8:TC * c8 + BDK],
                        start=(gi == 0 and c8 == 0),
                        stop=(gi == n_grp - 1 and c8 == 7),
                    )
                if gi == n_grp - 1:
                    emit_tiny_chain(s)

            def emit_tiny_chain(s):
                g_sb = wpool.tile([BDK, BDK], f32, name="g_sb", tag="g_sb",
                                  bufs=2)
                nc.vector.tensor_mul(g_sb[:], g_tiles[s][0:BDK, :], mk_t[:])
                m1_ps = pp.tile([BDK, K], f32, name="m1", tag="tiny", bufs=1)
                nc.tensor.matmul(m1_ps[:], g_sb[:], wq_t[:], start=True,
                                 stop=True)
                m1_sb = wpool.tile([BDK, K], f32, name="m1_sb", tag="m1_sb",
                                   bufs=2)
                nc.scalar.copy(m1_sb[:], m1_ps[:])
                st_ps = pp.tile([BDK, K], f32, name="st", tag="tiny", bufs=1)
                nc.tensor.matmul(st_ps[:], wk_t[:], m1_sb[:], start=True,
                                 stop=True)
                esl = slice(K * s, K * (s + 1))
                if have_bias:
                    eb_sb = wpool.tile([BDK, K], f32, name="eb_sb", tag="eb")
                    nc.vector.tensor_add(eb_sb[:], st_ps[:], corr_t[:, esl])
                    nc.scalar.activation(exp_all[:, esl], eb_sb[:], Exp)
                else:
                    nc.scalar.activation(exp_all[:, esl], st_ps[:], Exp)

            pend = []

            def gen_A(s):
                if s == 2:
                    emit_const_dmas()
                for gi in range(n_grp):
                    pend.append((s, gi, emit_transposes(s, gi)))
                    # just-in-time prefetch, interleaved with compute emission
                    while state["loaded"] < min(s + 2, S):
                        load_xtile(state["loaded"])
                        state["loaded"] += 1
                    if len(pend) > 2:
                        ps, pgi, xt = pend.pop(0)
                        emit_gram(ps, pgi, xt)
                    yield
                state["a_left"] -= 1

            def drain_pend():
                while pend:
                    ps, pgi, xt = pend.pop(0)
                    emit_gram(ps, pgi, xt)

            # --- per-tile segment sums -> reciprocal (sliding window) ---
            def emit_C_tile(t):
                idxs = [i for i, (ss, tt) in enumerate(used) if tt == t]
                seg_ps = pp.tile([BDK + 1, BDK], f32, name=f"seg{t}",
                                 tag="gsb", bufs=2)
                for n, i in enumerate(idxs):
                    s = used[i][0]
                    nc.tensor.matmul(
                        seg_ps[0:BDK, 0:K],
                        sel_t[:, BDK * i:BDK * (i + 1)],
                        exp_all[:, K * s:K * (s + 1)],
                        start=(n == 0), stop=(n == len(idxs) - 1),
                    )
                seg_sb = wpool.tile([BDK, K], f32, name="seg_sb",
                                    tag="seg_sb")
                nc.vector.tensor_scalar_max(
                    seg_sb[:], seg_ps[0:BDK, 0:K], 1e-30)
                nc.vector.reciprocal(inv_t[:, K * t:K * (t + 1)],
                                     seg_sb[:])
                state["recip_done"].add(t)

            # --- phase D: pipelined per stack ---
            state["nrelu"] = 0
            attn_tiles = {}

            def emit_attn_chain(s):
                state["attn_done"].add(s)
                idxs = [i for i, (ss, tt) in enumerate(used) if ss == s]
                invb_ps = pp.tile([BDK, K], f32, name="invb", tag="tiny",
                                  bufs=1)
                for n, i in enumerate(idxs):
                    t = used[i][1]
                    nc.tensor.matmul(
                        invb_ps[:],
                        selt_t[:, BDK * i:BDK * (i + 1)],
                        inv_t[:, K * t:K * (t + 1)],
                        start=(n == 0), stop=(n == len(idxs) - 1),
                    )
                attn_sb = wpool.tile([BDK, K], bf16, name="attn_sb",
                                     tag="attn_c", bufs=2)
                nc.vector.tensor_mul(attn_sb[:], exp_all[:, K * s:K * (s + 1)],
                                     invb_ps[:])
                # block-diag mask applied in one op via a 0-stride broadcast
                attn_bd = wpool.tile([BDK, BDK], bf16, name="attn_bd",
                                     tag="attn", bufs=2)
                attn_rep = attn_sb[:].unsqueeze(1).to_broadcast((BDK, BD, K))
                nc.vector.tensor_mul(attn_bd[:], attn_rep, mkb_t[:])
                attn_tiles[s] = attn_bd

            def gen_D(s):
                b_ps = pp.tile([BDK + 1, BDK], f32, name="b_ps", tag="gsb",
                               bufs=2)
                nc.tensor.matmul(b_ps[:], wv_t[:], attn_tiles.pop(s)[:],
                                 start=True, stop=True)
                # pad B to 128 weight cols: enables PE fast weight load on
                # the output matmuls (extra PSUM rows are never read)
                b_sb = wpool.tile([BDK + 1, WPAD], bf16, name="b_sb", tag="B",
                                  bufs=2)
                nc.vector.tensor_add(b_sb[:, 0:BDK], b_ps[:], ia_t[:])
                nc.gpsimd.memset(b_sb[:, BDK:WPAD], 0.0)
                nxt = s + 1
                if (nxt < S and nxt not in state["attn_done"]
                        and all(t in state["recip_done"]
                                for t in tiles_of[nxt])):
                    emit_attn_chain(nxt)  # overlaps this stack's matmuls

                osb = opool.tile([BDK, HW], bf16, name="osb", tag="osb",
                                 bufs=3)
                xt_ = x_tiles[s]
                tail = state["a_left"] == 0 and state["d_after_a"] >= 2
                if state["a_left"] == 0:
                    state["d_after_a"] += 1
                yield
                for oc in range(HW // O_CH):
                    sl = slice(O_CH * oc, O_CH * (oc + 1))
                    o_ps = pp.tile([WPAD, O_CH], f32, name="o_ps", tag="ops",
                                   bufs=2)
                    nc.tensor.matmul(o_ps[:], b_sb[:], xt_[:, sl],
                                     start=True, stop=True)
                    on_act = (oc % 2 == 1) if tail else (oc % 4 == 3)
                    if on_act:
                        nc.scalar.activation(osb[:, sl], o_ps[0:BDK, :], Relu)
                    else:
                        nc.vector.tensor_scalar_max(osb[:, sl],
                                                    o_ps[0:BDK, :], 0.0)
                    state["nrelu"] += 1
                    if oc == 3:
                        # first half of the store goes out as soon as its
                        # relus land: earlier store flow, shorter tail
                        seng = nc.sync if (s < S // 2 or s >= S - 2) \
                            else nc.gpsimd
                        hw2 = HW // 2
                        seng.dma_start(
                            y_d.ap()[:, s * HW:s * HW + hw2], osb[:, 0:hw2])
                    if oc % 2 == 1 and oc < 7:
                        yield
                # early stores ride the otherwise-empty sync HWDGE ring
                # (overlapping the SWDGE loads); late stores ride the SWDGE
                # ring once the loads have drained
                hw2 = HW // 2
                if s < S // 2 or s >= S - 2:
                    nc.sync.dma_start(
                        y_d.ap()[:, s * HW + hw2:(s + 1) * HW],
                        osb[:, hw2:HW])
                else:
                    nc.gpsimd.dma_start(
                        y_d.ap()[:, s * HW + hw2:(s + 1) * HW],
                        osb[:, hw2:HW])

            # --- schedule: sliding window — emit tile t's recip one stack
            # after its last contribution (the exp has drained by then), and
            # emit D stacks trailing A by >=2 stacks, so the in-order engine
            # streams never hit a semaphore stall (stalls re-throttle the PE
            # clock via HAM). Loads and stores interleave on the ring. ---
            # Each D's output chunks are zipped BETWEEN the next A stack's
            # transpose groups so a relu-paced output chunk never stalls the
            # in-order PE queue.
            state["interleave"] = False
            d_next = 0
            dg = None
            dstate = {"dg": None}

            def start_D_if_ready(s):
                nonlocal d_next
                if (dstate["dg"] is None and d_next < S and d_next <= s - 2
                        and all(t in state["recip_done"]
                                for t in tiles_of[d_next])):
                    state["interleave"] = True
                    if d_next not in state["attn_done"]:
                        emit_attn_chain(d_next)
                    dstate["dg"] = gen_D(d_next)
                    next(dstate["dg"])
                    d_next += 1

            def step_D():
                if dstate["dg"] is not None:
                    try:
                        next(dstate["dg"])
                    except StopIteration:
                        dstate["dg"] = None

            for s in range(S):
                for _ in gen_A(s):
                    step_D()
                for t in range(T):
                    if ready[t] == s - 1:
                        emit_C_tile(t)
                start_D_if_ready(s)
            drain_pend()
            while dstate["dg"] is not None:
                step_D()
            for t in range(T):
                if t not in state["recip_done"]:
                    emit_C_tile(t)
            while d_next < S:
                if d_next not in state["attn_done"]:
                    emit_attn_chain(d_next)
                for _ in gen_D(d_next):
                    pass
                d_next += 1

    nc.compile()
    return nc


def _get_compiled(S, T, have_bias: bool, used: tuple):
    key = (S, T, have_bias, used)
    if key not in _cache:
        _cache[key] = _build(S, T, have_bias, used)
    return _cache[key]


def _bd7(m: np.ndarray) -> np.ndarray:
    out = np.zeros((BDK, BDK), dtype=np.float32)
    for j in range(BD):
        out[K * j:K * (j + 1), K * j:K * (j + 1)] = m
    return out


W_TILE = 14  # persons per tile window: smaller -> tighter A->D pipeline


def _plan(ids: np.ndarray):
    """Split persons into N_CORES contiguous chunks at imgid boundaries.
    All cores run the same compiled program padded to S stacks, so the
    objective is minimizing the max chunk (greedy furthest-reach at the
    minimal per-core stack budget)."""
    change = np.flatnonzero(np.diff(ids)) + 1
    allb = np.concatenate([[0], change, [P_TOTAL]]).astype(np.int64)

    Smin = math.ceil(P_TOTAL / (N_CORES * BD))
    for S in range(Smin, Smin + 4):
        bounds = [0]
        for _ in range(N_CORES):
            a = bounds[-1]
            cand = allb[(allb >= a) & (allb <= a + BD * S)]
            bounds.append(int(cand[-1]))
            if bounds[-1] == P_TOTAL:
                break
        while len(bounds) < N_CORES + 1:
            bounds.append(bounds[-1])
        if bounds[-1] == P_TOTAL:
            return bounds, S
    raise AssertionError("no feasible core split")


def _prepare(inputs: dict):
    import ml_dtypes
    nbf16 = ml_dtypes.bfloat16

    x = np.asarray(inputs["kpt_feat"], dtype=np.float32).reshape(
        P_TOTAL, K, HW)
    ids = np.asarray(inputs["imgid"]).astype(np.int64)
    Wq = np.asarray(inputs["Wq"], np.float32)
    Wk = np.asarray(inputs["Wk"], np.float32)
    Wv = np.asarray(inputs["Wv"], np.float32)
    bq = np.asarray(inputs["bq"], np.float32)
    bk = np.asarray(inputs["bk"], np.float32)
    bv = np.asarray(inputs["bv"], np.float32)

    bounds, S = _plan(ids)
    P_pad = S * BD
    # per-core local group index per person; each group goes to the tile
    # whose person-position window [t*W_TILE, (t+1)*W_TILE) contains its
    # first person (aligned across cores so tile-ready stacks line up),
    # spilling forward if a tile's 7 slot rows fill up
    lgs = []
    gslots = []  # per core: group -> (tile, slot row)
    T = 1
    for ci in range(N_CORES):
        a, b = bounds[ci], bounds[ci + 1]
        if b > a:
            _, starts, lg = np.unique(ids[a:b], return_index=True,
                                      return_inverse=True)
        else:
            starts = np.zeros((0,), np.int64)
            lg = np.zeros((0,), np.int64)
        cnt: dict = {}
        tprev = 0
        gs = []
        for p0 in starts:
            t = max(tprev, int(p0) // W_TILE)
            while cnt.get(t, 0) >= BD:
                t += 1
            gs.append((t, cnt.get(t, 0)))
            cnt[t] = cnt.get(t, 0) + 1
            tprev = t
        lgs.append(lg)
        gslots.append(gs)
    # compress tile ids to a dense global range (windows can be empty)
    present = sorted({t for gs in gslots for (t, _) in gs})
    remap = {t: i for i, t in enumerate(present)}
    gslots = [[(remap[t], r) for (t, r) in gs] for gs in gslots]
    T = max(1, len(present))

    wq_col = np.zeros((BDK, K), np.float32)
    for j in range(BD):
        wq_col[K * j:K * (j + 1), :] = Wq.T / NORM
    wkt_bd = _bd7(Wk.T.astype(np.float32))
    wv_aug = np.zeros((BDK, BDK + 1), np.float32)
    wv_aug[:, :BDK] = _bd7(Wv)
    for j in range(BD):
        wv_aug[K * j:K * (j + 1), BDK] = bv
    wv_aug = wv_aug.astype(nbf16)
    id119 = np.eye(BDK, dtype=np.float32).astype(nbf16)
    iaug = np.zeros((BDK + 1, BDK), np.float32)
    iaug[:BDK, :BDK] = np.eye(BDK, dtype=np.float32)
    bdmask = _bd7(np.ones((K, K), np.float32))

    have_bias = bool(np.any(bq) or np.any(bk))
    if have_bias:
        xsum = x.sum(axis=2)
        qx = xsum @ Wq.T
        kx = xsum @ Wk.T
        corr_all = (bk[None, :, None] * qx[:, None, :]
                    + bq[None, None, :] * kx[:, :, None]
                    + HW * (bq[None, None, :] * bk[None, :, None])) / NORM
        corr_all = corr_all.astype(np.float32)  # [P, m, i]
    else:
        corr_all = None

    xb = x.astype(nbf16)

    # selector tensors per core: group g -> slot row (g % GPT) of tile
    # (g // GPT); padding persons have all-zero selector rows (their exp
    # contributes nowhere and their attn comes out zero)
    eye = np.eye(K, dtype=np.float32)
    sels = []
    newpos_all = []
    used_set = set()
    for ci in range(N_CORES):
        a, b = bounds[ci], bounds[ci + 1]
        pc = b - a
        newpos_all.append(np.arange(pc))
        lg = lgs[ci]
        gs = gslots[ci]
        sel = np.zeros((S, T, BDK, BDK), np.float32)
        for pos in range(pc):
            s, j = divmod(pos, BD)
            t, lgi = gs[int(lg[pos])]
            sel[s, t, K * j:K * (j + 1), K * lgi:K * (lgi + 1)] = eye
            used_set.add((s, t))
        sels.append(sel)
    used = tuple(sorted(used_set))

    in_maps = []
    for ci in range(N_CORES):
        a, b = bounds[ci], bounds[ci + 1]
        pc = b - a
        # partition-major x: [120, S*HW]; row 119 = ones (residual fold)
        np_ = newpos_all[ci]
        rows = np.zeros((P_pad, K, HW), dtype=nbf16)
        if pc:
            rows[np_] = xb[a:b]
        rows = rows.reshape(P_pad * K, HW)
        arr3 = np.zeros((S, BDK + 1, HW), dtype=nbf16)
        arr3[:, :BDK] = rows.reshape(S, BDK, HW)
        arr3[:, BDK] = 1.0
        xs = np.ascontiguousarray(
            arr3.transpose(1, 0, 2).reshape(BDK + 1, S * HW))
        sel = sels[ci]
        su = np.stack([sel[s, t] for (s, t) in used])  # [U, 119, 119]
        sel_pack = su.transpose(1, 0, 2).reshape(BDK, len(used) * BDK)
        selt_pack = su.transpose(2, 0, 1).reshape(BDK, len(used) * BDK)
        m = {
            "x": xs,
            "wq_col": wq_col,
            "wkt_bd": wkt_bd,
            "wv_aug": wv_aug,
            "id119": id119,
            "iaug": iaug,
            "bdmask": bdmask,
            "bdmaskb": bdmask.astype(nbf16),
            "sel": np.ascontiguousarray(sel_pack).astype(nbf16),
            "selT": np.ascontiguousarray(selt_pack).astype(nbf16),
        }
        if have_bias:
            corr_col = np.zeros((BDK, K * S), np.float32)
            if pc:
                cpad = np.zeros((P_pad, K, K), np.float32)
                cpad[np_] = corr_all[a:b]
                for s in range(S):
                    for j in range(BD):
                        corr_col[K * j:K * (j + 1), K * s:K * (s + 1)] = \
                            cpad[BD * s + j]
            m["corr_col"] = corr_col
        in_maps.append(m)
    return in_maps, bounds, newpos_all, (S, T), have_bias, used


def _gather(results, bounds, newpos_all, S):
    out = np.empty((P_TOTAL, K, 64, 64), dtype=np.float32)
    for ci in range(N_CORES):
        a, b = bounds[ci], bounds[ci + 1]
        pc = b - a
        if pc:
            y = np.asarray(results[ci]["y"], dtype=np.float32)  # [119, S*HW]
            y = y.reshape(BDK, S, HW).transpose(1, 0, 2).reshape(
                S * BD, K, 64, 64)
            out[a:b] = y[newpos_all[ci]]
    return out


def _run(inputs: dict, trace: bool = False):
    _ensure_path()
    from concourse.bass_utils import run_bass_kernel_spmd

    in_maps, bounds, newpos_all, (S, T), have_bias, used = \
        _prepare(inputs)
    nc = _get_compiled(S, T, have_bias, used)
    res = run_bass_kernel_spmd(nc, in_maps, list(range(N_CORES)), trace=trace)
    return _gather(res.results, bounds, newpos_all, S), res


def kernel(**inputs) -> np.ndarray:
    out, _ = _run(inputs, trace=False)
    return out


# revision 62
# speedup vs baseline: 1.1320x; 1.0079x over previous
"""Trainium2 Bass kernel for nn_JointRelationModule (self-contained).

Math (per person p; softmax is segment-softmax over persons within an imgid
group, elementwise over the (K,K) score entries):
    q = Wq x + bq ; k = Wk x + bk ; v = Wv x + bv      (1x1 conv over K=17)
    S_p = q_p k_p^T / 64
    attn = segment-softmax over persons
    out = relu(attn_p @ v_p + x_p)

Device formulation (heavy ops bf16 on the PE, block-column layouts):
  - Stack BD=7 persons as [119, hw]. Per stack: G = x x^T via PE transpose +
    accumulating matmuls (bf16, f32 PSUM).
  - scores^T in block-column layout [119, 17] via a masked-Gram matmul chain
    (block-diag mask kills cross-person terms), so no gather/scatter DMAs.
  - Segment softmax via per-stack selector matmuls into group-slot tiles,
    reciprocal, selector-transpose broadcast back; all partition-aligned.
  - Output: B = blockdiag((attn Wv)^T) + I with an av row appended; the
    residual and v-bias ride along x_aug (all-ones row), so each output chunk
    is one matmul + one relu. B is zero-padded to 128 weight columns so the
    PE fast-weight-load path kicks in. Stored bf16, host upcasts.

Data movement: x and y live in a partition-major layout [120, S*hw]. All bulk
x loads / y stores ride the gpsimd (SWDGE) ring: its descriptors spread
evenly over all 16 SDMA engines, unlike the HWDGE (sync/scalar) rings which
concentrate on engines 0-6. Small constants ride the otherwise-idle HWDGE
rings. Stack 0's load is column-chunked so the first transposes start early.

Sharding: data-parallel over persons at imgid group boundaries (8 cores),
weights replicated. Host casts x to bf16 (halves load bytes); output comes
back bf16 (halves store bytes). Tolerance 2e-2; measured error ~5e-3.
"""

import math
import sys

import numpy as np

K = 17
HW = 4096  # 64*64
P_TOTAL = 512
N_CORES = 8
NORM = 64.0
BD = 7          # persons per stack
BDK = BD * K    # 119
O_CH = 512      # output chunk cols (one PSUM bank of f32)
WPAD = 128      # output-matmul weight cols padded for fast weight load

_cache: dict = {}


def _ensure_path():
    try:
        import concourse.bass  # noqa: F401
    except ImportError:
        for p in ("/opt/trn_rl_repo", "/root/.axon_site/_ro/trn_rl_repo"):
            if p not in sys.path:
                sys.path.insert(0, p)
        import concourse.bass  # noqa: F401


def _build(S: int, T: int, have_bias: bool, used: tuple):
    """Builds + compiles the per-core SPMD Bass program.

    Sliding-window softmax: group-slot tiles hold only a few groups each, so
    tile t's reciprocal is ready as soon as its last contributing stack's exp
    is done; output stacks trail input stacks by the tile span (~3 stacks).
    Loads and stores interleave continuously on the SWDGE ring."""
    _ensure_path()
    import concourse.bacc as bacc
    import concourse.mybir as mybir
    import concourse.tile as tile

    f32 = mybir.dt.float32
    bf16 = mybir.dt.bfloat16
    Exp = mybir.ActivationFunctionType.Exp
    Relu = mybir.ActivationFunctionType.Relu

    U = len(used)
    tiles_of = {s: sorted({t for (ss, t) in used if ss == s})
                for s in range(S)}
    ready = {t: max(ss for (ss, tt) in used if tt == t)
             for t in range(T)}

    nc = bacc.Bacc(
        "TRN2",
        target_bir_lowering=False,
        debug=False,
        enable_asserts=False,
        num_devices=N_CORES,
    )

    x_d = nc.dram_tensor("x", [BDK + 1, S * HW], bf16, kind="ExternalInput")
    wq_d = nc.dram_tensor("wq_col", [BDK, K], f32, kind="ExternalInput")
    wk_d = nc.dram_tensor("wkt_bd", [BDK, BDK], f32, kind="ExternalInput")
    wv_d = nc.dram_tensor("wv_aug", [BDK, BDK + 1], bf16, kind="ExternalInput")
    id_d = nc.dram_tensor("id119", [BDK, BDK], bf16, kind="ExternalInput")
    ia_d = nc.dram_tensor("iaug", [BDK + 1, BDK], f32, kind="ExternalInput")
    mk_d = nc.dram_tensor("bdmask", [BDK, BDK], f32, kind="ExternalInput")
    mkb_d = nc.dram_tensor("bdmaskb", [BDK, BDK], bf16, kind="ExternalInput")
    sel_d = nc.dram_tensor("sel", [BDK, U * BDK], bf16, kind="ExternalInput")
    selt_d = nc.dram_tensor("selT", [BDK, U * BDK], bf16,
                            kind="ExternalInput")
    if have_bias:
        corr_d = nc.dram_tensor("corr_col", [BDK, K * S], f32,
                                kind="ExternalInput")
    y_d = nc.dram_tensor("y", [BDK, S * HW], bf16, kind="ExternalOutput")

    G_CH = 1024          # x cols per transpose group
    n_grp = HW // G_CH   # 4 groups per stack

    with tile.TileContext(nc) as tc:
        with (
            nc.allow_low_precision(reason="bf16 softmax ok at 2e-2 tol"),
            tc.tile_pool(name="xpool", bufs=1) as xpool,
            tc.tile_pool(name="cpool", bufs=1) as cpool,
            tc.tile_pool(name="wpool", bufs=2) as wpool,
            tc.tile_pool(name="opool", bufs=2) as opool,
            tc.tile_pool(name="pp", bufs=2, space="PSUM") as pp,
        ):
            # --- tiny phase-A constants first (id_t gates every transpose);
            # consts ride the HWDGE rings, bulk x/y rides the SWDGE ring ---
            id_t = cpool.tile([BDK, BDK], bf16, name="id_t", tag="id")
            mk_t = cpool.tile([BDK, BDK], f32, name="mk_t", tag="mk")
            wq_t = cpool.tile([BDK, K], f32, name="wq_t", tag="wq")
            wk_t = cpool.tile([BDK, BDK], f32, name="wk_t", tag="wk")
            nc.sync.dma_start(id_t[:], id_d.ap())

            # HAM warm-up: ~2.5us of dummy matmuls on id_t while waiting for
            # the first x chunk, so the PE clock is at 2.4GHz (not the cold
            # 1.2GHz) when the real transposes start; the result is unread
            warm_ps = pp.tile([BDK, BDK], f32, name="warm", tag="tiny",
                              bufs=1)
            for _ in range(22):
                nc.tensor.matmul(warm_ps[:], id_t[:], id_t[:],
                                 start=True, stop=True)

            x_tiles = []  # per stack

            def load_xtile(s):
                xt_ = xpool.tile([BDK + 1, HW], bf16, name=f"xp{s}",
                                 tag=f"xp{s}")
                base = s * HW
                # chunked so transposes can start on a partial tile: the
                # first chunk's arrival, not the whole tile's, gates compute
                nch = 4
                cw = HW // nch
                for ci_ in range(nch):
                    nc.gpsimd.dma_start(
                        xt_[:, cw * ci_:cw * (ci_ + 1)],
                        x_d.ap()[:, base + cw * ci_:base + cw * (ci_ + 1)])
                x_tiles.append(xt_)

            nc.scalar.dma_start(mk_t[:], mk_d.ap())
            nc.sync.dma_start(wq_t[:], wq_d.ap())
            nc.scalar.dma_start(wk_t[:], wk_d.ap())
            load_xtile(0)
            if have_bias:
                corr_t = cpool.tile([BDK, K * S], f32, name="corr_t",
                                    tag="corr")
                nc.scalar.dma_start(corr_t[:], corr_d.ap())

            # bulkier constants: tiles declared now, DMAs emitted mid-phase-A
            # (the framework coalesces DMA waits into a cumulative counter, so
            # anything emitted before the first transpose delays it)
            wv_t = cpool.tile([BDK, BDK + 1], bf16, name="wv_t", tag="wv")
            ia_t = cpool.tile([BDK + 1, BDK], f32, name="ia_t", tag="ia")
            sel_t = cpool.tile([BDK, U * BDK], bf16, name="sel_t", tag="sel")
            selt_t = cpool.tile([BDK, U * BDK], bf16, name="selt_t",
                                tag="selt")
            mkb_t = cpool.tile([BDK, BDK], bf16, name="mkb_t", tag="mkb")

            def emit_const_dmas():
                nc.sync.dma_start(wv_t[:], wv_d.ap())
                nc.scalar.dma_start(ia_t[:], ia_d.ap())
                nc.sync.dma_start(mkb_t[:], mkb_d.ap())
                nc.scalar.dma_start(sel_t[:], sel_d.ap())
                nc.sync.dma_start(selt_t[:], selt_d.ap())

            exp_all = cpool.tile([BDK, K * S], bf16, name="exp_all", tag="exp")
            inv_t = cpool.tile([BDK, K * T], bf16, name="inv_t", tag="inv")

            # --- phase A: transpose -> gram -> scores^T -> exp, skewed ---
            # PSUM tags (8 banks): big=tp/o_ps x4, gsb=g/seg/b x2, tiny x2
            TC = BDK + 1         # 120: chunk col stride (4B-aligned in PSUM)
            state = {"ncopy": 0, "loaded": 1, "a_left": S, "d_after_a": 0,
                     "interleave": False, "recip_done": set(),
                     "attn_done": set()}
            g_tiles = {}

            # engine split: PSUM-reading elementwise work can only run on
            # DVE/ACT (GpSimd has no PSUM access). During A/D overlap, keep
            # A's copies on ACT and D's relus mostly on DVE so the in-order
            # engine queues don't cross-block; use both engines otherwise.
            def emit_transposes(s, gi):
                xt_ = x_tiles[s]
                tp = pp.tile([128, 8 * TC], bf16, name="tp", tag="tp",
                             bufs=3)
                for c8 in range(8):
                    col = G_CH * gi + 128 * c8
                    nc.tensor.transpose(
                        tp[:, TC * c8:TC * c8 + BDK],
                        xt_[0:BDK, col:col + 128], id_t[:],
                    )
                xt = wpool.tile([128, 8 * TC], bf16, name="xt", tag="xt",
                                bufs=4)
                if state["interleave"] or state["ncopy"] % 2 == 1:
                    nc.scalar.copy(xt[:], tp[:])
                else:
                    nc.vector.tensor_copy(xt[:], tp[:])
                state["ncopy"] += 1
                return xt

            def emit_gram(s, gi, xt):
                if s not in g_tiles:
                    g_tiles[s] = pp.tile([BDK + 1, BDK], f32, name=f"g{s}",
                                         tag="gsb", bufs=2)
                g_ps = g_tiles[s]
                for c8 in range(8):
                    nc.tensor.matmul(
                        g_ps[0:BDK, :], xt[:, TC * c8:TC * c8 + BDK],
                        xt[:, TC * c8:TC * c8 + BDK],
                        start=(gi == 0 and c8 == 0),
                        stop=(gi == n_grp - 1 and c8 == 7),
                    )
                if gi == n_grp - 1:
                    emit_tiny_chain(s)

            def emit_tiny_chain(s):
                g_sb = wpool.tile([BDK, BDK], f32, name="g_sb", tag="g_sb",
                                  bufs=2)
                nc.vector.tensor_mul(g_sb[:], g_tiles[s][0:BDK, :], mk_t[:])
                m1_ps = pp.tile([BDK, K], f32, name="m1", tag="tiny", bufs=1)
                nc.tensor.matmul(m1_ps[:], g_sb[:], wq_t[:], start=True,
                                 stop=True)
                m1_sb = wpool.tile([BDK, K], f32, name="m1_sb", tag="m1_sb",
                                   bufs=2)
                nc.scalar.copy(m1_sb[:], m1_ps[:])
                st_ps = pp.tile([BDK, K], f32, name="st", tag="tiny", bufs=1)
                nc.tensor.matmul(st_ps[:], wk_t[:], m1_sb[:], start=True,
                                 stop=True)
                esl = slice(K * s, K * (s + 1))
                if have_bias:
                    eb_sb = wpool.tile([BDK, K], f32, name="eb_sb", tag="eb")
                    nc.vector.tensor_add(eb_sb[:], st_ps[:], corr_t[:, esl])
                    nc.scalar.activation(exp_all[:, esl], eb_sb[:], Exp)
                else:
                    nc.scalar.activation(exp_all[:, esl], st_ps[:], Exp)

            pend = []

            def gen_A(s):
                if s == 2:
                    emit_const_dmas()
                for gi in range(n_grp):
                    pend.append((s, gi, emit_transposes(s, gi)))
                    # just-in-time prefetch, interleaved with compute emission
                    while state["loaded"] < min(s + 2, S):
                        load_xtile(state["loaded"])
                        state["loaded"] += 1
                    if len(pend) > 2:
                        ps, pgi, xt = pend.pop(0)
                        emit_gram(ps, pgi, xt)
                    yield
                state["a_left"] -= 1

            def drain_pend():
                while pend:
                    ps, pgi, xt = pend.pop(0)
                    emit_gram(ps, pgi, xt)

            # --- per-tile segment sums -> reciprocal (sliding window) ---
            def emit_C_tile(t):
                idxs = [i for i, (ss, tt) in enumerate(used) if tt == t]
                seg_ps = pp.tile([BDK + 1, BDK], f32, name=f"seg{t}",
                                 tag="gsb", bufs=2)
                for n, i in enumerate(idxs):
                    s = used[i][0]
                    nc.tensor.matmul(
                        seg_ps[0:BDK, 0:K],
                        sel_t[:, BDK * i:BDK * (i + 1)],
                        exp_all[:, K * s:K * (s + 1)],
                        start=(n == 0), stop=(n == len(idxs) - 1),
                    )
                seg_sb = wpool.tile([BDK, K], f32, name="seg_sb",
                                    tag="seg_sb")
                nc.vector.tensor_scalar_max(
                    seg_sb[:], seg_ps[0:BDK, 0:K], 1e-30)
                nc.vector.reciprocal(inv_t[:, K * t:K * (t + 1)],
                                     seg_sb[:])
                state["recip_done"].add(t)

            # --- phase D: pipelined per stack ---
            state["nrelu"] = 0
            attn_tiles = {}

            def emit_attn_chain(s):
                state["attn_done"].add(s)
                idxs = [i for i, (ss, tt) in enumerate(used) if ss == s]
                invb_ps = pp.tile([BDK, K], f32, name="invb", tag="tiny",
                                  bufs=1)
                for n, i in enumerate(idxs):
                    t = used[i][1]
                    nc.tensor.matmul(
                        invb_ps[:],
                        selt_t[:, BDK * i:BDK * (i + 1)],
                        inv_t[:, K * t:K * (t + 1)],
                        start=(n == 0), stop=(n == len(idxs) - 1),
                    )
                attn_sb = wpool.tile([BDK, K], bf16, name="attn_sb",
                                     tag="attn_c", bufs=2)
                nc.vector.tensor_mul(attn_sb[:], exp_all[:, K * s:K * (s + 1)],
                                     invb_ps[:])
                # block-diag mask applied in one op via a 0-stride broadcast
                attn_bd = wpool.tile([BDK, BDK], bf16, name="attn_bd",
                                     tag="attn", bufs=2)
                attn_rep = attn_sb[:].unsqueeze(1).to_broadcast((BDK, BD, K))
                nc.vector.tensor_mul(attn_bd[:], attn_rep, mkb_t[:])
                attn_tiles[s] = attn_bd

            def gen_D(s):
                b_ps = pp.tile([BDK + 1, BDK], f32, name="b_ps", tag="gsb",
                               bufs=2)
                nc.tensor.matmul(b_ps[:], wv_t[:], attn_tiles.pop(s)[:],
                                 start=True, stop=True)
                # pad B to 128 weight cols: enables PE fast weight load on
                # the output matmuls (extra PSUM rows are never read)
                b_sb = wpool.tile([BDK + 1, WPAD], bf16, name="b_sb", tag="B",
                                  bufs=2)
                nc.vector.tensor_add(b_sb[:, 0:BDK], b_ps[:], ia_t[:])
                nc.gpsimd.memset(b_sb[:, BDK:WPAD], 0.0)
                nxt = s + 1
                if (nxt < S and nxt not in state["attn_done"]
                        and all(t in state["recip_done"]
                                for t in tiles_of[nxt])):
                    emit_attn_chain(nxt)  # overlaps this stack's matmuls

                osb = opool.tile([BDK, HW], bf16, name="osb", tag="osb",
                                 bufs=3)
                xt_ = x_tiles[s]
                tail = state["a_left"] == 0 and state["d_after_a"] >= 2
                if state["a_left"] == 0:
                    state["d_after_a"] += 1
                yield
                for oc in range(HW // O_CH):
                    sl = slice(O_CH * oc, O_CH * (oc + 1))
                    o_ps = pp.tile([WPAD, O_CH], f32, name="o_ps", tag="ops",
                                   bufs=2)
                    nc.tensor.matmul(o_ps[:], b_sb[:], xt_[:, sl],
                                     start=True, stop=True)
                    on_act = (oc % 2 == 1) if tail else (oc % 4 == 3)
                    if on_act:
                        nc.scalar.activation(osb[:, sl], o_ps[0:BDK, :], Relu)
                    else:
                        nc.vector.tensor_scalar_max(osb[:, sl],
                                                    o_ps[0:BDK, :], 0.0)
                    state["nrelu"] += 1
                    if oc == 3:
                        # first half of the store goes out as soon as its
                        # relus land: earlier store flow, shorter tail
                        seng = nc.sync if (s < S // 2 or s >= S - 3) \
                            else nc.gpsimd
                        hw2 = HW // 2
                        seng.dma_start(
                            y_d.ap()[:, s * HW:s * HW + hw2], osb[:, 0:hw2])
                    if oc % 2 == 1 and oc < 7:
                        yield
                # early stores ride the otherwise-empty sync HWDGE ring
                # (overlapping the SWDGE loads); late stores ride the SWDGE
                # ring once the loads have drained
                hw2 = HW // 2
                if s < S // 2 or s >= S - 3:
                    nc.sync.dma_start(
                        y_d.ap()[:, s * HW + hw2:(s + 1) * HW],
                        osb[:, hw2:HW])
                else:
                    nc.gpsimd.dma_start(
                        y_d.ap()[:, s * HW + hw2:(s + 1) * HW],
                        osb[:, hw2:HW])

            # --- schedule: sliding window — emit tile t's recip one stack
            # after its last contribution (the exp has drained by then), and
            # emit D stacks trailing A by >=2 stacks, so the in-order engine
            # streams never hit a semaphore stall (stalls re-throttle the PE
            # clock via HAM). Loads and stores interleave on the ring. ---
            # Each D's output chunks are zipped BETWEEN the next A stack's
            # transpose groups so a relu-paced output chunk never stalls the
            # in-order PE queue.
            state["interleave"] = False
            d_next = 0
            dg = None
            dstate = {"dg": None}

            def start_D_if_ready(s):
                nonlocal d_next
                if (dstate["dg"] is None and d_next < S and d_next <= s - 2
                        and all(t in state["recip_done"]
                                for t in tiles_of[d_next])):
                    state["interleave"] = True
                    if d_next not in state["attn_done"]:
                        emit_attn_chain(d_next)
                    dstate["dg"] = gen_D(d_next)
                    next(dstate["dg"])
                    d_next += 1

            def step_D():
                if dstate["dg"] is not None:
                    try:
                        next(dstate["dg"])
                    except StopIteration:
                        dstate["dg"] = None

            for s in range(S):
                for _ in gen_A(s):
                    step_D()
                for t in range(T):
                    if ready[t] == s - 1:
                        emit_C_tile(t)
                start_D_if_ready(s)
            drain_pend()
            while dstate["dg"] is not None:
                step_D()
            for t in range(T):
                if t not in state["recip_done"]:
                    emit_C_tile(t)
            while d_next < S:
                if d_next not in state["attn_done"]:
                    emit_attn_chain(d_next)
                for _ in gen_D(d_next):
                    pass
                d_next += 1

    nc.compile()
    return nc


def _get_compiled(S, T, have_bias: bool, used: tuple):
    key = (S, T, have_bias, used)
    if key not in _cache:
        _cache[key] = _build(S, T, have_bias, used)
    return _cache[key]


def _bd7(m: np.ndarray) -> np.ndarray:
    out = np.zeros((BDK, BDK), dtype=np.float32)
    for j in range(BD):
        out[K * j:K * (j + 1), K * j:K * (j + 1)] = m
    return out


W_TILE = 14  # persons per tile window: smaller -> tighter A->D pipeline


def _plan(ids: np.ndarray):
    """Split persons into N_CORES contiguous chunks at imgid boundaries.
    All cores run the same compiled program padded to S stacks, so the
    objective is minimizing the max chunk (greedy furthest-reach at the
    minimal per-core stack budget)."""
    change = np.flatnonzero(np.diff(ids)) + 1
    allb = np.concatenate([[0], change, [P_TOTAL]]).astype(np.int64)

    Smin = math.ceil(P_TOTAL / (N_CORES * BD))
    for S in range(Smin, Smin + 4):
        bounds = [0]
        for _ in range(N_CORES):
            a = bounds[-1]
            cand = allb[(allb >= a) & (allb <= a + BD * S)]
            bounds.append(int(cand[-1]))
            if bounds[-1] == P_TOTAL:
                break
        while len(bounds) < N_CORES + 1:
            bounds.append(bounds[-1])
        if bounds[-1] == P_TOTAL:
            return bounds, S
    raise AssertionError("no feasible core split")


def _prepare(inputs: dict):
    import ml_dtypes
    nbf16 = ml_dtypes.bfloat16

    x = np.asarray(inputs["kpt_feat"], dtype=np.float32).reshape(
        P_TOTAL, K, HW)
    ids = np.asarray(inputs["imgid"]).astype(np.int64)
    Wq = np.asarray(inputs["Wq"], np.float32)
    Wk = np.asarray(inputs["Wk"], np.float32)
    Wv = np.asarray(inputs["Wv"], np.float32)
    bq = np.asarray(inputs["bq"], np.float32)
    bk = np.asarray(inputs["bk"], np.float32)
    bv = np.asarray(inputs["bv"], np.float32)

    bounds, S = _plan(ids)
    P_pad = S * BD
    # per-core local group index per person; each group goes to the tile
    # whose person-position window [t*W_TILE, (t+1)*W_TILE) contains its
    # first person (aligned across cores so tile-ready stacks line up),
    # spilling forward if a tile's 7 slot rows fill up
    lgs = []
    gslots = []  # per core: group -> (tile, slot row)
    T = 1
    for ci in range(N_CORES):
        a, b = bounds[ci], bounds[ci + 1]
        if b > a:
            _, starts, lg = np.unique(ids[a:b], return_index=True,
                                      return_inverse=True)
        else:
            starts = np.zeros((0,), np.int64)
            lg = np.zeros((0,), np.int64)
        cnt: dict = {}
        tprev = 0
        gs = []
        for p0 in starts:
            t = max(tprev, int(p0) // W_TILE)
            while cnt.get(t, 0) >= BD:
                t += 1
            gs.append((t, cnt.get(t, 0)))
            cnt[t] = cnt.get(t, 0) + 1
            tprev = t
        lgs.append(lg)
        gslots.append(gs)
    # compress tile ids to a dense global range (windows can be empty)
    present = sorted({t for gs in gslots for (t, _) in gs})
    remap = {t: i for i, t in enumerate(present)}
    gslots = [[(remap[t], r) for (t, r) in gs] for gs in gslots]
    T = max(1, len(present))

    wq_col = np.zeros((BDK, K), np.float32)
    for j in range(BD):
        wq_col[K * j:K * (j + 1), :] = Wq.T / NORM
    wkt_bd = _bd7(Wk.T.astype(np.float32))
    wv_aug = np.zeros((BDK, BDK + 1), np.float32)
    wv_aug[:, :BDK] = _bd7(Wv)
    for j in range(BD):
        wv_aug[K * j:K * (j + 1), BDK] = bv
    wv_aug = wv_aug.astype(nbf16)
    id119 = np.eye(BDK, dtype=np.float32).astype(nbf16)
    iaug = np.zeros((BDK + 1, BDK), np.float32)
    iaug[:BDK, :BDK] = np.eye(BDK, dtype=np.float32)
    bdmask = _bd7(np.ones((K, K), np.float32))

    have_bias = bool(np.any(bq) or np.any(bk))
    if have_bias:
        xsum = x.sum(axis=2)
        qx = xsum @ Wq.T
        kx = xsum @ Wk.T
        corr_all = (bk[None, :, None] * qx[:, None, :]
                    + bq[None, None, :] * kx[:, :, None]
                    + HW * (bq[None, None, :] * bk[None, :, None])) / NORM
        corr_all = corr_all.astype(np.float32)  # [P, m, i]
    else:
        corr_all = None

    xb = x.astype(nbf16)

    # selector tensors per core: group g -> slot row (g % GPT) of tile
    # (g // GPT); padding persons have all-zero selector rows (their exp
    # contributes nowhere and their attn comes out zero)
    eye = np.eye(K, dtype=np.float32)
    sels = []
    newpos_all = []
    used_set = set()
    for ci in range(N_CORES):
        a, b = bounds[ci], bounds[ci + 1]
        pc = b - a
        newpos_all.append(np.arange(pc))
        lg = lgs[ci]
        gs = gslots[ci]
        sel = np.zeros((S, T, BDK, BDK), np.float32)
        for pos in range(pc):
            s, j = divmod(pos, BD)
            t, lgi = gs[int(lg[pos])]
            sel[s, t, K * j:K * (j + 1), K * lgi:K * (lgi + 1)] = eye
            used_set.add((s, t))
        sels.append(sel)
    used = tuple(sorted(used_set))

    in_maps = []
    for ci in range(N_CORES):
        a, b = bounds[ci], bounds[ci + 1]
        pc = b - a
        # partition-major x: [120, S*HW]; row 119 = ones (residual fold)
        np_ = newpos_all[ci]
        rows = np.zeros((P_pad, K, HW), dtype=nbf16)
        if pc:
            rows[np_] = xb[a:b]
        rows = rows.reshape(P_pad * K, HW)
        arr3 = np.zeros((S, BDK + 1, HW), dtype=nbf16)
        arr3[:, :BDK] = rows.reshape(S, BDK, HW)
        arr3[:, BDK] = 1.0
        xs = np.ascontiguousarray(
            arr3.transpose(1, 0, 2).reshape(BDK + 1, S * HW))
        sel = sels[ci]
        su = np.stack([sel[s, t] for (s, t) in used])  # [U, 119, 119]
        sel_pack = su.transpose(1, 0, 2).reshape(BDK, len(used) * BDK)
        selt_pack = su.transpose(2, 0, 1).reshape(BDK, len(used) * BDK)
        m = {
            "x": xs,
            "wq_col": wq_col,
            "wkt_bd": wkt_bd,
            "wv_aug": wv_aug,
            "id119": id119,
            "iaug": iaug,
            "bdmask": bdmask,
            "bdmaskb": bdmask.astype(nbf16),
            "sel": np.ascontiguousarray(sel_pack).astype(nbf16),
            "selT": np.ascontiguousarray(selt_pack).astype(nbf16),
        }
        if have_bias:
            corr_col = np.zeros((BDK, K * S), np.float32)
            if pc:
                cpad = np.zeros((P_pad, K, K), np.float32)
                cpad[np_] = corr_all[a:b]
                for s in range(S):
                    for j in range(BD):
                        corr_col[K * j:K * (j + 1), K * s:K * (s + 1)] = \
                            cpad[BD * s + j]
            m["corr_col"] = corr_col
        in_maps.append(m)
    return in_maps, bounds, newpos_all, (S, T), have_bias, used


def _gather(results, bounds, newpos_all, S):
    out = np.empty((P_TOTAL, K, 64, 64), dtype=np.float32)
    for ci in range(N_CORES):
        a, b = bounds[ci], bounds[ci + 1]
        pc = b - a
        if pc:
            y = np.asarray(results[ci]["y"], dtype=np.float32)  # [119, S*HW]
            y = y.reshape(BDK, S, HW).transpose(1, 0, 2).reshape(
                S * BD, K, 64, 64)
            out[a:b] = y[newpos_all[ci]]
    return out


def _run(inputs: dict, trace: bool = False):
    _ensure_path()
    from concourse.bass_utils import run_bass_kernel_spmd

    in_maps, bounds, newpos_all, (S, T), have_bias, used = \
        _prepare(inputs)
    nc = _get_compiled(S, T, have_bias, used)
    res = run_bass_kernel_spmd(nc, in_maps, list(range(N_CORES)), trace=trace)
    return _gather(res.results, bounds, newpos_all, S), res


def kernel(**inputs) -> np.ndarray:
    out, _ = _run(inputs, trace=False)
    return out
